# revision 1
# baseline (speedup 1.0000x reference)
"""AdaConvBlock Trainium2 kernel: 8-core data-parallel (2 batch elems/core).

Per core (b=2, C=384, L=4096):
  LN1 -> adaLN modulate -> SLConv (1024-tap depthwise conv via four-step
  matmul FFT, N=4608=128x36, batch pair packed as complex) + D-skip ->
  gated residual -> LN2 -> modulate -> pointwise MLP (gelu) -> gated residual.

FFT: Cooley-Tukey twiddles folded into 36 per-n2 stationary matrices ->
pure matmul FFT, no twiddle pointwise work. PSUM is managed in explicit
2KB-bank slots (one matmul output per slot, has_written gives per-slot
overwrite/accumulate semantics).
"""
import os
import sys

sys.path.insert(0, "/opt/trn_rl_repo")

import numpy as np
import ml_dtypes

import concourse.bass as bass
import concourse.bacc as bacc
import concourse.tile as tile
from concourse import mybir
from concourse.bass_utils import run_bass_kernel_spmd

F32 = mybir.dt.float32
BF16 = mybir.dt.bfloat16
AX = mybir.AluOpType
AF = mybir.ActivationFunctionType

B, C, L = 16, 384, 4096
NCORES = 8
BPC = B // NCORES
CT = 3
NCH = 8
LCH = 512
N, N1, N2 = 4608, 128, 36
KLEN = 1024
SS = 512
NG = 8
GS = 48
NSUB = 16
EPS = 1e-5
DECAY = 2.0
NS, KS = 6, 32

_last_results = None


def _make_consts():
    k1 = np.arange(N1)
    n1 = np.arange(N1)
    W128 = np.exp(-2j * np.pi * np.outer(k1, n1) / N1)
    W36 = np.exp(-2j * np.pi * np.outer(np.arange(N2), np.arange(N2)) / N2)
    fA = np.zeros((N2, N1, N1), complex)
    iA = np.zeros((N2, N1, N1), complex)
    for n2 in range(N2):
        M = np.exp(-2j * np.pi * n2 * k1 / N)[:, None] * W128      # [k1,n1]
        fA[n2] = M.T                                               # lhsT [n1,k1]
        IA = (np.exp(2j * np.pi * n2 * k1 / N)[None, :] * np.conj(W128).T) / N
        iA[n2] = IA.T                                              # lhsT [k1,n1]

    def blockdiag3(Mx):
        out = np.zeros((108, 108), complex)
        for s in range(3):
            out[s * 36:(s + 1) * 36, s * 36:(s + 1) * 36] = Mx
        return out

    fB = blockdiag3(W36.T)
    iB = blockdiag3(np.conj(W36).T)

    def bf(x):
        return np.ascontiguousarray(x).astype(ml_dtypes.bfloat16)

    return {
        "fA_re": bf(fA.real), "fA_im": bf(fA.imag), "fA_imn": bf(-fA.imag),
        "iA_re": bf(iA.real), "iA_im": bf(iA.imag), "iA_imn": bf(-iA.imag),
        "fB_re": bf(fB.real), "fB_im": bf(fB.imag), "fB_imn": bf(-fB.imag),
        "iB_re": bf(iB.real), "iB_im": bf(iB.imag), "iB_imn": bf(-iB.imag),
        "ident": bf(np.eye(128)),
        "ones_bf": bf(np.ones((128, 128))),
    }


# bank-slot offset for per-n2 FFT matmul outputs: 10 slots of 48 per 2KB bank
def _n2off(n2):
    return (n2 // 10) * 512 + (n2 % 10) * 48


def build_graph():
    nc = bacc.Bacc(None)

    x_e = nc.declare_dram_parameter("x", [BPC, C, L], F32, isOutput=False)
    tc_e = nc.declare_dram_parameter("t_cond", [BPC, C // 3, L], F32, isOutput=False)
    ker_e = nc.declare_dram_parameter("kernels", [NS, 1, C, KS], F32, isOutput=False)
    d_e = nc.declare_dram_parameter("DT", [128, CT], F32, isOutput=False)
    adawT_e = nc.declare_dram_parameter("ada_wT", [C // 3, 6 * C], BF16, isOutput=False)
    adab_e = nc.declare_dram_parameter("ada_bT", [128, 18], F32, isOutput=False)
    w1T_e = nc.declare_dram_parameter("w1T", [C, C], BF16, isOutput=False)
    b1_e = nc.declare_dram_parameter("b1T", [128, CT], F32, isOutput=False)
    w2T_e = nc.declare_dram_parameter("w2T", [C, C], BF16, isOutput=False)
    b2_e = nc.declare_dram_parameter("b2T", [128, CT], F32, isOutput=False)
    cshapes = {
        "fA_re": [N2, N1, N1], "fA_im": [N2, N1, N1], "fA_imn": [N2, N1, N1],
        "iA_re": [N2, N1, N1], "iA_im": [N2, N1, N1], "iA_imn": [N2, N1, N1],
        "fB_re": [108, 108], "fB_im": [108, 108], "fB_imn": [108, 108],
        "iB_re": [108, 108], "iB_im": [108, 108], "iB_imn": [108, 108],
        "ident": [128, 128], "ones_bf": [128, 128],
    }
    cst = {nm: nc.declare_dram_parameter(nm, shp, BF16, isOutput=False)
           for nm, shp in cshapes.items()}
    out_e = nc.declare_dram_parameter("out", [BPC, C, L], F32, isOutput=True)

    ymod_d = nc.dram_tensor("ymod", [BPC, C, N], BF16)
    yconv_d = nc.dram_tensor("yconv", [BPC, C, N], BF16)
    kpad_d = nc.dram_tensor("kpad", [C, 29 * N2], BF16)

    MM = nc.tensor.matmul

    with tile.TileContext(nc) as tc, \
         nc.allow_low_precision(reason="bf16 datapath, fp32 psum accumulation"), \
         tc.tile_pool(name="sing", bufs=1) as sing:
        if True:
            zc = sing.tile([128, 1], F32)
            nc.vector.memset(zc, 0.0)
            nc.const_aps.aps[(F32, 0.0)] = zc[:, :]
            ec = sing.tile([128, 1], F32)
            nc.vector.memset(ec, EPS)
            nc.const_aps.aps[(F32, EPS)] = ec[:, :]
            sb = {}
            for qi, nm in enumerate(("fA_re", "fA_im", "fA_imn", "iA_re",
                                     "iA_im", "iA_imn")):
                t = sing.tile([N1, N2, N1], BF16, tag=nm)
                deng = nc.sync if qi % 2 == 0 else nc.gpsimd
                deng.dma_start(out=t, in_=cst[nm].rearrange("a b c -> b a c"))
                sb[nm] = t
            for nm in ("fB_re", "fB_im", "fB_imn", "iB_re", "iB_im", "iB_imn"):
                t = sing.tile([108, 108], BF16, tag=nm)
                nc.sync.dma_start(out=t, in_=cst[nm][:, :])
                sb[nm] = t
            ident = sing.tile([128, 128], BF16)
            nc.sync.dma_start(out=ident, in_=cst["ident"][:, :])
            ident32 = sing.tile([128, 128], F32)
            nc.scalar.activation(ident32, ident, AF.Copy)
            ones_bf = sing.tile([128, 128], BF16)
            nc.sync.dma_start(out=ones_bf, in_=cst["ones_bf"][:, :])
            adawT = sing.tile([128, 18, 128], BF16)
            nc.sync.dma_start(out=adawT,
                              in_=adawT_e.rearrange("k (c o) -> k c o", c=18))
            adab = sing.tile([128, 18], F32)
            nc.sync.dma_start(out=adab, in_=adab_e[:, :])
            w1T = sing.tile([128, CT, C], BF16)
            nc.sync.dma_start(out=w1T,
                              in_=w1T_e.rearrange("(a k) o -> k a o", k=128))
            w2T = sing.tile([128, CT, C], BF16)
            nc.sync.dma_start(out=w2T,
                              in_=w2T_e.rearrange("(a k) o -> k a o", k=128))
            b1c = sing.tile([128, CT], F32)
            nc.sync.dma_start(out=b1c, in_=b1_e[:, :])
            b2c = sing.tile([128, CT], F32)
            nc.sync.dma_start(out=b2c, in_=b2_e[:, :])
            dcol = sing.tile([128, CT], F32)
            nc.sync.dma_start(out=dcol, in_=d_e[:, :])

            # ---------------- kernel build -> kpad_d ----------------
            import os as _os
            _STAGES = int(_os.environ.get("KSTAGES", "4"))
            with tc.tile_pool(name="kb", bufs=2) as kb:
                zpad20 = kb.tile([128, 20], BF16)
                nc.vector.memset(zpad20, 0.0)
                offs = [0, 32, 64, 128, 256, 512]
                for ct in range(CT):
                    kdec = kb.tile([128, KLEN], F32, tag="kdec")
                    for i in range(NS):
                        rep = 2 ** max(0, i - 1)
                        kraw = kb.tile([128, KS], F32, tag="kraw")
                        nc.sync.dma_start(
                            out=kraw,
                            in_=ker_e[i, 0, ct * 128:(ct + 1) * 128, :])
                        ksrc = kraw[:, :]
                        src3 = bass.AP(tensor=ksrc.tensor, offset=ksrc.offset,
                                       ap=[ksrc.ap[0], ksrc.ap[1], [0, rep]])
                        dst = kdec[:, offs[i]:offs[i] + KS * rep].rearrange(
                            "p (t r) -> p t r", r=rep)
                        nc.scalar.activation(dst, src3, AF.Copy,
                                             scale=float(DECAY ** (NS - i - 1)))
                    ksq = kb.tile([128, KLEN], BF16, tag="ksq")
                    ssum = kb.tile([128, 1], F32, tag="ssum")
                    nc.scalar.activation(ksq, kdec, AF.Square, accum_out=ssum)
                    sd = kb.tile([128, 1], F32, tag="sdk")
                    nc.scalar.activation(sd, ssum, AF.Sqrt)
                    rn = kb.tile([128, 1], F32, tag="rnk")
                    nc.vector.reciprocal(rn, sd)
                    knb = kb.tile([128, KLEN], BF16, tag="knb")
                    nc.scalar.activation(knb, kdec, AF.Copy, scale=rn[:, 0:1])
                    nc.gpsimd.dma_start(
                        out=kpad_d[ct * 128:(ct + 1) * 128, 0:KLEN], in_=knb)
                    nc.gpsimd.dma_start(
                        out=kpad_d[ct * 128:(ct + 1) * 128, KLEN:], in_=zpad20)

            tc.strict_bb_all_engine_barrier()
            # ---------------- phase 1: LN1 + modulate -> ymod -------
            if _STAGES >= 2:
             with tc.tile_pool(name="p1", bufs=3) as p1, \
                 tc.tile_pool(name="p1p", bufs=1, space="PSUM") as p1p, \
                 tc.tile_pool(name="p1q", bufs=3, space="PSUM") as p1q:
                zpadN = p1.tile([128, N - L], BF16, tag="zpadN")
                nc.vector.memset(zpadN, 0.0)
                for b in range(BPC):
                    for ct in range(CT):
                        nc.gpsimd.dma_start(
                            out=ymod_d[b, ct * 128:(ct + 1) * 128, L:N],
                            in_=zpadN)
                for b in range(BPC):
                    for ch in range(NCH):
                        l0 = ch * LCH
                        xf = p1.tile([128, CT, LCH], F32, tag="xf")
                        for ct in range(CT):
                            dq = nc.sync if ct != 1 else nc.scalar
                            dq.dma_start(
                                out=xf[:, ct, :],
                                in_=x_e[b, ct * 128:(ct + 1) * 128, l0:l0 + LCH])
                        xb = p1.tile([128, CT, LCH], BF16, tag="xb")
                        x2 = p1.tile([128, CT, LCH], BF16, tag="x2")
                        for ct in range(CT):
                            if ct == 0:
                                nc.vector.tensor_copy(xb[:, ct, :], xf[:, ct, :])
                            else:
                                nc.scalar.activation(xb[:, ct, :], xf[:, ct, :],
                                                     AF.Copy)
                            nc.gpsimd.tensor_mul(x2[:, ct, :], xb[:, ct, :],
                                                 xb[:, ct, :])
                        s1p = p1p.tile([128, LCH], F32, tag="s1p")
                        s2p = p1p.tile([128, LCH], F32, tag="s2p")
                        for ct in range(CT):
                            MM(s1p, ones_bf, xb[:, ct, :],
                               start=(ct == 0), stop=(ct == CT - 1))
                        for ct in range(CT):
                            MM(s2p, ones_bf, x2[:, ct, :],
                               start=(ct == 0), stop=(ct == CT - 1))
                        mu = p1.tile([128, LCH], BF16, tag="mu")
                        ex2 = p1.tile([128, LCH], F32, tag="ex2")
                        nc.scalar.activation(mu, s1p, AF.Copy, scale=1.0 / C)
                        nc.scalar.activation(ex2, s2p, AF.Copy, scale=1.0 / C)
                        musq = p1.tile([128, LCH], F32, tag="musq")
                        nc.gpsimd.tensor_mul(musq, mu, mu)
                        var = p1.tile([128, LCH], F32, tag="var")
                        nc.vector.tensor_sub(var, ex2, musq)
                        sd_ = p1.tile([128, LCH], F32, tag="sd_")
                        nc.scalar.activation(sd_, var, AF.Sqrt, bias=EPS)
                        inv = p1.tile([128, LCH], BF16, tag="inv")
                        nc.vector.reciprocal(inv, sd_)
                        muinv = p1.tile([128, LCH], BF16, tag="muinv")
                        nc.vector.tensor_mul(muinv, mu, inv)
                        tcf = p1.tile([128, LCH], F32, tag="tcf")
                        nc.sync.dma_start(out=tcf, in_=tc_e[b, :, l0:l0 + LCH])
                        tsil = p1.tile([128, LCH], BF16, tag="tsil")
                        nc.scalar.activation(tsil, tcf, AF.Silu)
                        ym = p1.tile([128, CT, LCH], BF16, tag="ym")
                        for ct in range(CT):
                            adp = p1q.tile([128, 2, LCH], F32, tag="adp")
                            MM(adp[:, 0, :], adawT[:, ct, :], tsil,
                               start=True, stop=True)
                            MM(adp[:, 1, :], adawT[:, 3 + ct, :], tsil,
                               start=True, stop=True)
                            m1 = p1.tile([128, LCH], BF16, tag=f"m1_{ct}")
                            nc.gpsimd.tensor_mul(m1, xb[:, ct, :], inv)
                            z = p1.tile([128, LCH], BF16, tag=f"z_{ct}")
                            nc.gpsimd.tensor_sub(z, m1, muinv)
                            t_ = p1.tile([128, LCH], BF16, tag=f"t_{ct}")
                            nc.vector.scalar_tensor_tensor(
                                t_, adp[:, 1, :], adab[:, 3 + ct:4 + ct],
                                z, op0=AX.add, op1=AX.mult)
                            y1 = p1.tile([128, LCH], BF16, tag=f"y1_{ct}")
                            nc.gpsimd.tensor_add(y1, t_, z)
                            nc.vector.scalar_tensor_tensor(
                                ym[:, ct, :], adp[:, 0, :],
                                adab[:, ct:ct + 1], y1, op0=AX.add, op1=AX.add)
                        for ct in range(CT):
                            nc.gpsimd.dma_start(
                                out=ymod_d[b, ct * 128:(ct + 1) * 128,
                                           l0:l0 + LCH],
                                in_=ym[:, ct, :])

            tc.strict_bb_all_engine_barrier()
            # ---------------- phase 2: kernel FFT + conv FFT --------
            if _STAGES >= 3:
             with tc.tile_pool(name="p2", bufs=1) as p2, \
                 tc.tile_pool(name="p2in", bufs=2) as p2in, \
                 tc.tile_pool(name="p2p", bufs=1, space="PSUM") as p2p:

                H2 = [(0, 20), (20, 16)]    # n2 halves (start, count)
                HS = [(0, 8), (8, 8)]        # subgroup halves

                def pair(h):
                    t = "a" if h == 0 else "b"
                    pre = p2p.tile([128, 1024], F32, tag=f"p{t}_re")
                    pim = p2p.tile([128, 1024], F32, tag=f"p{t}_im")
                    return pre, pim

                def unscr_half(dst, psrc, h, eng):
                    # psrc [128,1024]: 2 banks of 10 slots x 48 -> dst ch-major
                    d3 = dst.rearrange("p (c n) -> p c n", n=N2)
                    st, cnt = H2[h]
                    s5 = psrc.rearrange("p (bk r) -> p bk r", bk=2)[
                        :, :, 0:480].rearrange("p bk (sl c) -> p bk sl c",
                                               sl=10)
                    act = eng is nc.scalar
                    if cnt == 20:
                        o = d3[:, :, st:st + 20].rearrange(
                            "p c (bk sl) -> p bk sl c", bk=2)
                        if act:
                            eng.activation(o, s5[:, :, :, 0:48], AF.Copy)
                        else:
                            eng.tensor_copy(o, s5[:, :, :, 0:48])
                    else:
                        o1 = d3[:, :, st:st + 10].rearrange("p c n -> p n c")
                        o2 = d3[:, :, st + 10:st + 16].rearrange(
                            "p c n -> p n c")
                        if act:
                            eng.activation(o1, s5[:, 0, :, 0:48], AF.Copy)
                            eng.activation(o2, s5[:, 1, 0:6, 0:48], AF.Copy)
                        else:
                            eng.tensor_copy(o1, s5[:, 0, :, 0:48])
                            eng.tensor_copy(o2, s5[:, 1, 0:6, 0:48])

                def f1_half(pre, pim, h, zr, zi, real):
                    st, cnt = H2[h]
                    for jx in range(cnt):
                        n2 = st + jx
                        off = (jx // 10) * 512 + (jx % 10) * 48
                        if real:
                            MM(pre[:, off:off + GS], sb["fA_re"][0:29, n2, :],
                               zr[:, :, n2], start=True, stop=True)
                            MM(pim[:, off:off + GS], sb["fA_im"][0:29, n2, :],
                               zr[:, :, n2], start=True, stop=True)
                        else:
                            MM(pre[:, off:off + GS], sb["fA_re"][:, n2, :],
                               zr[:, :, n2], start=True, stop=False)
                            MM(pim[:, off:off + GS], sb["fA_im"][:, n2, :],
                               zr[:, :, n2], start=True, stop=False)
                            MM(pre[:, off:off + GS], sb["fA_imn"][:, n2, :],
                               zi[:, :, n2], start=False, stop=True)
                            MM(pim[:, off:off + GS], sb["fA_re"][:, n2, :],
                               zi[:, :, n2], start=False, stop=True)

                def i4_half(pre, pim, h, vr3, vi3):
                    st, cnt = H2[h]
                    for jx in range(cnt):
                        n2 = st + jx
                        off = (jx // 10) * 512 + (jx % 10) * 48
                        MM(pre[:, off:off + GS], sb["iA_re"][:, n2, :],
                           vr3[:, :, n2], start=True, stop=False)
                        MM(pim[:, off:off + GS], sb["iA_im"][:, n2, :],
                           vr3[:, :, n2], start=True, stop=False)
                        MM(pre[:, off:off + GS], sb["iA_imn"][:, n2, :],
                           vi3[:, :, n2], start=False, stop=True)
                        MM(pim[:, off:off + GS], sb["iA_re"][:, n2, :],
                           vi3[:, :, n2], start=False, stop=True)

                def t_half(pre, pim, h, inre, inim):
                    st, _ = HS[h]
                    for s in range(st, st + 8):
                        off = ((s - st) // 4) * 512 + ((s - st) % 4) * 128
                        isl = slice(s * 108, (s + 1) * 108)
                        MM(pre[:108, off:off + 128], inre[:, isl], ident32,
                           is_transpose=True, start=True, stop=True)
                        MM(pim[:108, off:off + 128], inim[:, isl], ident32,
                           is_transpose=True, start=True, stop=True)

                def tb_half(pre, pim, h, inre, inim):
                    st, _ = HS[h]
                    for s in range(st, st + 8):
                        off = ((s - st) // 4) * 512 + ((s - st) % 4) * 108
                        isl = slice(s * 128, (s + 1) * 128)
                        MM(pre[:, off:off + 108], inre[:108, isl],
                           ident32[:108, :108], is_transpose=True, start=True,
                           stop=True)
                        MM(pim[:, off:off + 108], inim[:108, isl],
                           ident32[:108, :108], is_transpose=True, start=True,
                           stop=True)

                def d36_half(pre, pim, h, Bre, Bim, Bimn, inre, inim):
                    st, _ = HS[h]
                    for s in range(st, st + 8):
                        off = (s - st) * 128
                        sl = slice(s * 128, (s + 1) * 128)
                        MM(pre[:108, off:off + 128], Bre, inre[:, sl],
                           start=True, stop=False)
                        MM(pim[:108, off:off + 128], Bim, inre[:, sl],
                           start=True, stop=False)
                        MM(pre[:108, off:off + 128], Bimn, inim[:, sl],
                           start=False, stop=True)
                        MM(pim[:108, off:off + 128], Bre, inim[:, sl],
                           start=False, stop=True)

                def hcopy(dst, psrc, h, eng):
                    o = dst[:, h * 1024:(h + 1) * 1024]
                    if eng is nc.scalar:
                        eng.activation(o, psrc[:108, :], AF.Copy)
                    else:
                        eng.tensor_copy(o, psrc[:108, :])

                def vcopy_half(vflat, psrc, h, eng):
                    for bk in range(2):
                        o = vflat[:, (h * 8 + bk * 4) * 108:
                                  (h * 8 + bk * 4) * 108 + 432]
                        s_ = psrc[:, bk * 512:bk * 512 + 432]
                        if eng is nc.scalar:
                            eng.activation(o, s_, AF.Copy)
                        else:
                            eng.tensor_copy(o, s_)

                for g in range(NG):
                    c0 = g * GS
                    # ======== kernel FFT for this group ========
                    kz = p2in.tile([29, GS, N2], BF16, tag="kz")
                    nc.sync.dma_start(
                        out=kz, in_=kpad_d[c0:c0 + GS, :].rearrange(
                            "c (a b) -> a c b", b=N2))
                    S_re = p2.tile([128, GS * N2], F32, tag="S_re")
                    S_im = p2.tile([128, GS * N2], F32, tag="S_im")
                    for h in (0, 1):
                        pre, pim = pair(h)
                        f1_half(pre, pim, h, kz, None, True)
                        unscr_half(S_re, pre, h, nc.scalar)
                        unscr_half(S_im, pim, h, nc.vector)
                    ST_re = p2.tile([108, NSUB * 128], BF16, tag="ST_re")
                    ST_im = p2.tile([108, NSUB * 128], BF16, tag="ST_im")
                    for h in (0, 1):
                        pre, pim = pair(h)
                        t_half(pre, pim, h, S_re, S_im)
                        hcopy(ST_re, pre, h, nc.scalar)
                        hcopy(ST_im, pim, h, nc.vector)
                    kh_re = p2.tile([108, NSUB * 128], BF16, tag="kh_re")
                    kh_im = p2.tile([108, NSUB * 128], BF16, tag="kh_im")
                    for h in (0, 1):
                        pre, pim = pair(h)
                        d36_half(pre, pim, h, sb["fB_re"], sb["fB_im"],
                                 sb["fB_imn"], ST_re, ST_im)
                        hcopy(kh_re, pre, h, nc.scalar)
                        hcopy(kh_im, pim, h, nc.vector)

                    # ======== data FFT (batch pair packed complex) ========
                    z_re = p2in.tile([128, GS, N2], BF16, tag="z_re")
                    z_im = p2in.tile([128, GS, N2], BF16, tag="z_im")
                    nc.sync.dma_start(
                        out=z_re, in_=ymod_d[0, c0:c0 + GS, :].rearrange(
                            "c (a b) -> a c b", b=N2))
                    nc.sync.dma_start(
                        out=z_im, in_=ymod_d[1, c0:c0 + GS, :].rearrange(
                            "c (a b) -> a c b", b=N2))
                    S_re = p2.tile([128, GS * N2], F32, tag="S_re")
                    S_im = p2.tile([128, GS * N2], F32, tag="S_im")
                    for h in (0, 1):
                        pre, pim = pair(h)
                        f1_half(pre, pim, h, z_re, z_im, False)
                        unscr_half(S_re, pre, h, nc.scalar)
                        unscr_half(S_im, pim, h, nc.vector)
                    ST_re = p2.tile([108, NSUB * 128], BF16, tag="ST_re")
                    ST_im = p2.tile([108, NSUB * 128], BF16, tag="ST_im")
                    for h in (0, 1):
                        pre, pim = pair(h)
                        t_half(pre, pim, h, S_re, S_im)
                        hcopy(ST_re, pre, h, nc.scalar)
                        hcopy(ST_im, pim, h, nc.vector)
                    X_re = p2.tile([108, NSUB * 128], BF16, tag="X_re")
                    X_im = p2.tile([108, NSUB * 128], BF16, tag="X_im")
                    for h in (0, 1):
                        pre, pim = pair(h)
                        d36_half(pre, pim, h, sb["fB_re"], sb["fB_im"],
                                 sb["fB_imn"], ST_re, ST_im)
                        hcopy(X_re, pre, h, nc.scalar)
                        hcopy(X_im, pim, h, nc.vector)
                    # spectral multiply (per half to keep pipeline fine-grained)
                    Y_re = p2.tile([108, NSUB * 128], BF16, tag="Y_re")
                    Y_im = p2.tile([108, NSUB * 128], BF16, tag="Y_im")
                    q1 = p2.tile([108, NSUB * 128], BF16, tag="q1")
                    q2 = p2.tile([108, NSUB * 128], BF16, tag="q2")
                    for h in (0, 1):
                        sl = slice(h * 1024, (h + 1) * 1024)
                        nc.vector.tensor_mul(q1[:, sl], X_re[:, sl],
                                             kh_re[:, sl])
                        nc.gpsimd.tensor_mul(q2[:, sl], X_im[:, sl],
                                             kh_im[:, sl])
                        nc.gpsimd.tensor_sub(Y_re[:, sl], q1[:, sl],
                                             q2[:, sl])
                        nc.vector.tensor_mul(q1[:, sl], X_re[:, sl],
                                             kh_im[:, sl])
                        nc.gpsimd.tensor_mul(q2[:, sl], X_im[:, sl],
                                             kh_re[:, sl])
                        nc.vector.tensor_add(Y_im[:, sl], q1[:, sl],
                                             q2[:, sl])
                    U_re = p2.tile([108, NSUB * 128], F32, tag="U_re")
                    U_im = p2.tile([108, NSUB * 128], F32, tag="U_im")
                    for h in (0, 1):
                        pre, pim = pair(h)
                        d36_half(pre, pim, h, sb["iB_re"], sb["iB_im"],
                                 sb["iB_imn"], Y_re, Y_im)
                        hcopy(U_re, pre, h, nc.scalar)
                        hcopy(U_im, pim, h, nc.vector)
                    V_re = p2.tile([128, GS, N2], BF16, tag="V_re")
                    V_im = p2.tile([128, GS, N2], BF16, tag="V_im")
                    vr = V_re.rearrange("p a b -> p (a b)")
                    vi = V_im.rearrange("p a b -> p (a b)")
                    for h in (0, 1):
                        pre, pim = pair(h)
                        tb_half(pre, pim, h, U_re, U_im)
                        vcopy_half(vr, pre, h, nc.scalar)
                        vcopy_half(vi, pim, h, nc.vector)
                    yo_re = p2.tile([128, GS, N2], BF16, tag="yo_re")
                    yo_im = p2.tile([128, GS, N2], BF16, tag="yo_im")
                    yof_re = yo_re.rearrange("p a b -> p (a b)")
                    yof_im = yo_im.rearrange("p a b -> p (a b)")
                    for h in (0, 1):
                        pre, pim = pair(h)
                        i4_half(pre, pim, h, V_re, V_im)
                        unscr_half(yof_re, pre, h, nc.scalar)
                        unscr_half(yof_im, pim, h, nc.vector)
                    nc.gpsimd.dma_start(
                        out=yconv_d[0, c0:c0 + GS, :].rearrange(
                            "c (a b) -> a c b", b=N2), in_=yo_re)
                    nc.gpsimd.dma_start(
                        out=yconv_d[1, c0:c0 + GS, :].rearrange(
                            "c (a b) -> a c b", b=N2), in_=yo_im)

            tc.strict_bb_all_engine_barrier()
            # ---------------- phase 3 ------------------------------
            if _STAGES >= 4:
             with tc.tile_pool(name="p3", bufs=1) as p3, \
                 tc.tile_pool(name="p3p", bufs=1, space="PSUM") as p3p, \
                 tc.tile_pool(name="p3pm", bufs=2, space="PSUM") as p3pm:
                for ch in range(NCH):
                    for b in range(BPC):
                        l0 = ch * LCH
                        yc = p3.tile([128, CT, LCH], BF16, tag="yc3" + str(b))
                        ym3 = p3.tile([128, CT, LCH], BF16, tag="ym3" + str(b))
                        xb = p3.tile([128, CT, LCH], BF16, tag="xb3" + str(b))
                        for ct in range(CT):
                            xstg = p3.tile([128, LCH], F32, tag="xstg" + str(b))
                            nc.sync.dma_start(
                                out=xstg,
                                in_=x_e[b, ct * 128:(ct + 1) * 128, l0:l0 + LCH])
                            nc.sync.dma_start(
                                out=yc[:, ct, :],
                                in_=yconv_d[b, ct * 128:(ct + 1) * 128,
                                            SS + l0:SS + l0 + LCH])
                            nc.sync.dma_start(
                                out=ym3[:, ct, :],
                                in_=ymod_d[b, ct * 128:(ct + 1) * 128,
                                           l0:l0 + LCH])
                            nc.scalar.activation(xb[:, ct, :], xstg, AF.Copy)
                        tcf = p3.tile([128, LCH], F32, tag="tcf3" + str(b))
                        nc.sync.dma_start(out=tcf, in_=tc_e[b, :, l0:l0 + LCH])
                        tsil = p3.tile([128, LCH], BF16, tag="tsil3" + str(b))
                        nc.scalar.activation(tsil, tcf, AF.Silu)
                        # gate_tm chunks -> SBUF
                        gts = p3.tile([128, CT, LCH], BF16, tag="gts" + str(b))
                        for ct in range(CT):
                            adp3 = p3pm.tile([128, LCH], F32, tag="adp3")
                            MM(adp3, adawT[:, 6 + ct, :], tsil,
                               start=True, stop=True)
                            nc.vector.tensor_scalar(
                                gts[:, ct, :], adp3, adab[:, 6 + ct:7 + ct],
                                None, AX.add)
                        x1 = p3.tile([128, CT, LCH], BF16, tag="x1" + str(b))
                        x2t = p3.tile([128, CT, LCH], BF16, tag="x2t" + str(b))
                        for ct in range(CT):
                            s1 = p3.tile([128, LCH], BF16, tag=f"s1_{ct}_{b}")
                            nc.vector.scalar_tensor_tensor(
                                s1, ym3[:, ct, :], dcol[:, ct:ct + 1],
                                yc[:, ct, :], op0=AX.mult, op1=AX.add)
                            gt = p3.tile([128, LCH], BF16, tag=f"gt_{ct}_{b}")
                            nc.vector.tensor_mul(gt, gts[:, ct, :], s1)
                            nc.gpsimd.tensor_add(x1[:, ct, :], xb[:, ct, :], gt)
                            nc.gpsimd.tensor_mul(x2t[:, ct, :], x1[:, ct, :],
                                                 x1[:, ct, :])
                        s1p3 = p3p.tile([128, LCH], F32, tag="s1p3" + str(b))
                        s2p3 = p3p.tile([128, LCH], F32, tag="s2p3" + str(b))
                        for ct in range(CT):
                            MM(s1p3, ones_bf, x1[:, ct, :],
                               start=(ct == 0), stop=(ct == CT - 1))
                        for ct in range(CT):
                            MM(s2p3, ones_bf, x2t[:, ct, :],
                               start=(ct == 0), stop=(ct == CT - 1))
                        mu = p3.tile([128, LCH], BF16, tag="mu3" + str(b))
                        ex2 = p3.tile([128, LCH], BF16, tag="ex23" + str(b))
                        nc.scalar.activation(mu, s1p3, AF.Copy,
                                             scale=1.0 / C)
                        nc.scalar.activation(ex2, s2p3, AF.Copy,
                                             scale=1.0 / C)
                        musq = p3.tile([128, LCH], BF16, tag="musq3" + str(b))
                        nc.gpsimd.tensor_mul(musq, mu, mu)
                        var = p3.tile([128, LCH], BF16, tag="var3" + str(b))
                        nc.vector.tensor_sub(var, ex2, musq)
                        sd_ = p3.tile([128, LCH], BF16, tag="sd_3" + str(b))
                        nc.scalar.activation(sd_, var, AF.Sqrt, bias=EPS)
                        inv = p3.tile([128, LCH], BF16, tag="inv3" + str(b))
                        nc.vector.reciprocal(inv, sd_)
                        muinv = p3.tile([128, LCH], BF16, tag="muinv3" + str(b))
                        nc.vector.tensor_mul(muinv, mu, inv)
                        # ada chunks 9-14 -> SBUF
                        cms = p3.tile([128, 6, LCH], BF16, tag="cms" + str(b))
                        for oc in range(6):
                            adp3 = p3pm.tile([128, LCH], F32, tag="adp3")
                            MM(adp3, adawT[:, 9 + oc, :], tsil,
                               start=True, stop=True)
                            nc.scalar.activation(cms[:, oc, :], adp3, AF.Identity,
                                                 bias=adab[:, 9 + oc:10 + oc])
                        z2 = p3.tile([128, CT, LCH], BF16, tag="z2" + str(b))
                        for ct in range(CT):
                            m1 = p3.tile([128, LCH], BF16, tag=f"m13_{ct}_{b}")
                            nc.gpsimd.tensor_mul(m1, x1[:, ct, :], inv)
                            z = p3.tile([128, LCH], BF16, tag=f"z3_{ct}_{b}")
                            nc.gpsimd.tensor_sub(z, m1, muinv)
                            t_ = p3.tile([128, LCH], BF16, tag=f"t3_{ct}_{b}")
                            nc.vector.tensor_mul(t_, cms[:, 3 + ct, :], z)
                            y1 = p3.tile([128, LCH], BF16, tag=f"y13_{ct}_{b}")
                            nc.gpsimd.tensor_add(y1, t_, z)
                            nc.vector.tensor_add(z2[:, ct, :],
                                                 cms[:, ct, :], y1)
                        # MLP layer 1
                        h = p3.tile([128, CT, LCH], BF16, tag="h" + str(b))
                        for oc in range(CT):
                            hp = p3pm.tile([128, LCH], F32, tag="mlp")
                            for ct in range(CT):
                                MM(hp, w1T[:, ct, oc * 128:(oc + 1) * 128],
                                   z2[:, ct, :], start=(ct == 0),
                                   stop=(ct == CT - 1))
                            nc.scalar.activation(h[:, oc, :], hp, AF.Gelu,
                                                 bias=b1c[:, oc:oc + 1])
                        # gate_cm -> SBUF
                        gcs = p3.tile([128, CT, LCH], BF16, tag="gcs" + str(b))
                        for ct in range(CT):
                            adp3 = p3pm.tile([128, LCH], F32, tag="adp3")
                            MM(adp3, adawT[:, 15 + ct, :], tsil,
                               start=True, stop=True)
                            nc.vector.tensor_scalar(
                                gcs[:, ct, :], adp3, adab[:, 15 + ct:16 + ct],
                                None, AX.add)
                        for oc in range(CT):
                            mp = p3pm.tile([128, LCH], F32, tag="mlp")
                            for ct in range(CT):
                                MM(mp, w2T[:, ct, oc * 128:(oc + 1) * 128],
                                   h[:, ct, :], start=(ct == 0),
                                   stop=(ct == CT - 1))
                            mb = p3.tile([128, LCH], BF16, tag=f"mb_{oc}_{b}")
                            nc.scalar.activation(mb, mp, AF.Identity,
                                                 bias=b2c[:, oc:oc + 1])
                            gc = p3.tile([128, LCH], BF16, tag=f"gc_{oc}_{b}")
                            nc.vector.tensor_mul(gc, gcs[:, oc, :], mb)
                            ostg = p3.tile([128, LCH], F32, tag="ostg" + str(b))
                            nc.vector.tensor_add(ostg, x1[:, oc, :], gc)
                            nc.gpsimd.dma_start(
                                out=out_e[b, oc * 128:(oc + 1) * 128,
                                          l0:l0 + LCH],
                                in_=ostg)
    nc.finalize()
    return nc


def kernel(x, t_cond, kernels, D, ada_w, ada_b, w1, b1, w2, b2):
    global _last_results
    consts = _make_consts()
    nc = build_graph()
    shared = {
        "kernels": np.ascontiguousarray(kernels, dtype=np.float32),
        "DT": np.ascontiguousarray(
            np.asarray(D, np.float32).reshape(CT, 128).T),
        "ada_wT": np.ascontiguousarray(ada_w.T).astype(ml_dtypes.bfloat16),
        "ada_bT": np.ascontiguousarray(
            np.asarray(ada_b, np.float32).reshape(18, 128).T),
        "w1T": np.ascontiguousarray(w1.T).astype(ml_dtypes.bfloat16),
        "b1T": np.ascontiguousarray(
            np.asarray(b1, np.float32).reshape(CT, 128).T),
        "w2T": np.ascontiguousarray(w2.T).astype(ml_dtypes.bfloat16),
        "b2T": np.ascontiguousarray(
            np.asarray(b2, np.float32).reshape(CT, 128).T),
    }
    shared.update(consts)
    in_maps = []
    for i in range(NCORES):
        m = dict(shared)
        m["x"] = np.ascontiguousarray(x[i * BPC:(i + 1) * BPC], dtype=np.float32)
        m["t_cond"] = np.ascontiguousarray(t_cond[i * BPC:(i + 1) * BPC],
                                           dtype=np.float32)
        in_maps.append(m)
    trace = os.environ.get("KERNEL_TRACE", "0") == "1"
    res = run_bass_kernel_spmd(nc, in_maps, list(range(NCORES)), trace=trace)
    _last_results = res
    outs = [r["out"] if isinstance(r, dict) else r for r in res.results]
    return np.concatenate([np.asarray(o, dtype=np.float32).reshape(BPC, C, L)
                           for o in outs], axis=0)


if __name__ == "__main__":
    build_graph()
    print("graph built ok")



# revision 7
# speedup vs baseline: 1.1719x; 1.1719x over previous
"""AdaConvBlock Trainium2 kernel: 8-core data-parallel (2 batch elems/core).

Per core (b=2, C=384, L=4096):
  LN1 -> adaLN modulate -> SLConv (1024-tap depthwise conv via four-step
  matmul FFT, N=4608=128x36, batch pair packed as complex) + D-skip ->
  gated residual -> LN2 -> modulate -> pointwise MLP (gelu) -> gated residual.

FFT: Cooley-Tukey twiddles folded into 36 per-n2 stationary matrices ->
pure matmul FFT, no twiddle pointwise work. PSUM is managed in explicit
2KB-bank slots (one matmul output per slot, has_written gives per-slot
overwrite/accumulate semantics).
"""
import os
import sys

sys.path.insert(0, "/opt/trn_rl_repo")

import numpy as np
import ml_dtypes

import concourse.bass as bass
import concourse.bacc as bacc
import concourse.tile as tile
from concourse import mybir
from concourse.bass_utils import run_bass_kernel_spmd

F32 = mybir.dt.float32
BF16 = mybir.dt.bfloat16
AX = mybir.AluOpType
AF = mybir.ActivationFunctionType

B, C, L = 16, 384, 4096
NCORES = 8
BPC = B // NCORES
CT = 3
NCH = 8
LCH = 512
N, N1, N2 = 4608, 128, 36
KLEN = 1024
SS = 512
NG = 8
GS = 48
NSUB = 16
EPS = 1e-5
DECAY = 2.0
NS, KS = 6, 32

_last_results = None


def _make_consts():
    k1 = np.arange(N1)
    n1 = np.arange(N1)
    W128 = np.exp(-2j * np.pi * np.outer(k1, n1) / N1)
    W36 = np.exp(-2j * np.pi * np.outer(np.arange(N2), np.arange(N2)) / N2)
    fA = np.zeros((N2, N1, N1), complex)
    iA = np.zeros((N2, N1, N1), complex)
    for n2 in range(N2):
        M = np.exp(-2j * np.pi * n2 * k1 / N)[:, None] * W128      # [k1,n1]
        fA[n2] = M.T                                               # lhsT [n1,k1]
        IA = (np.exp(2j * np.pi * n2 * k1 / N)[None, :] * np.conj(W128).T) / N
        iA[n2] = IA.T                                              # lhsT [k1,n1]

    def blockdiag3(Mx):
        out = np.zeros((108, 108), complex)
        for s in range(3):
            out[s * 36:(s + 1) * 36, s * 36:(s + 1) * 36] = Mx
        return out

    fB = blockdiag3(W36.T)
    iB = blockdiag3(np.conj(W36).T)

    def bf(x):
        return np.ascontiguousarray(x).astype(ml_dtypes.bfloat16)

    return {
        "fA_re": bf(fA.real), "fA_im": bf(fA.imag), "fA_imn": bf(-fA.imag),
        "iA_re": bf(iA.real), "iA_im": bf(iA.imag), "iA_imn": bf(-iA.imag),
        "fB_re": bf(fB.real), "fB_im": bf(fB.imag), "fB_imn": bf(-fB.imag),
        "iB_re": bf(iB.real), "iB_im": bf(iB.imag), "iB_imn": bf(-iB.imag),
        "ident": bf(np.eye(128)),
        "ones_bf": bf(np.ones((128, 128))),
    }


def _make_khat(kernels):
    """Host-side kernel FFT: build the normalized multi-scale kernel, FFT to
    length N=4608, and lay out per group in the spectral-domain tile layout
    produced by the on-device fwd FFT: KH[g, c_loc*36+k2, s*128+k1] =
    FFT(k_c)[k1 + 128*k2] with c = 48g + 3s + c_loc."""
    ker = np.asarray(kernels, np.float64)  # (NS, 1, C, KS)
    klist = [np.repeat(ker[i, 0], 2 ** max(0, i - 1), axis=-1)
             * (DECAY ** (NS - i - 1)) for i in range(NS)]
    k = np.concatenate(klist, axis=-1)  # (C, KLEN)
    k = k / np.linalg.norm(k, axis=-1, keepdims=True)
    Ksp = np.fft.fft(k, N, axis=-1)  # (C, N)
    Kp = Ksp.reshape(C, N2, N1)                      # [c, k2, k1]
    Kp = Kp.reshape(NG, GS // 3, 3, N2, N1)          # [g, s, c_loc, k2, k1]
    Kp = Kp.transpose(0, 2, 3, 1, 4).reshape(NG, 108, NSUB * 128)

    def bf(x):
        return np.ascontiguousarray(x).astype(ml_dtypes.bfloat16)

    return bf(Kp.real), bf(Kp.imag)


# bank-slot offset for per-n2 FFT matmul outputs: 10 slots of 48 per 2KB bank
def _n2off(n2):
    return (n2 // 10) * 512 + (n2 % 10) * 48


def build_graph():
    nc = bacc.Bacc(None)

    x_e = nc.declare_dram_parameter("x", [BPC, C, L], F32, isOutput=False)
    tc_e = nc.declare_dram_parameter("t_cond", [BPC, C // 3, L], F32, isOutput=False)
    khre_e = nc.declare_dram_parameter("khre", [NG, 108, NSUB * 128], BF16,
                                       isOutput=False)
    khim_e = nc.declare_dram_parameter("khim", [NG, 108, NSUB * 128], BF16,
                                       isOutput=False)
    d_e = nc.declare_dram_parameter("DT", [128, CT], F32, isOutput=False)
    adawT_e = nc.declare_dram_parameter("ada_wT", [C // 3, 6 * C], BF16, isOutput=False)
    adab_e = nc.declare_dram_parameter("ada_bT", [128, 18], F32, isOutput=False)
    w1T_e = nc.declare_dram_parameter("w1T", [C, C], BF16, isOutput=False)
    b1_e = nc.declare_dram_parameter("b1T", [128, CT], F32, isOutput=False)
    w2T_e = nc.declare_dram_parameter("w2T", [C, C], BF16, isOutput=False)
    b2_e = nc.declare_dram_parameter("b2T", [128, CT], F32, isOutput=False)
    cshapes = {
        "fA_re": [N2, N1, N1], "fA_im": [N2, N1, N1], "fA_imn": [N2, N1, N1],
        "iA_re": [N2, N1, N1], "iA_im": [N2, N1, N1], "iA_imn": [N2, N1, N1],
        "fB_re": [108, 108], "fB_im": [108, 108], "fB_imn": [108, 108],
        "iB_re": [108, 108], "iB_im": [108, 108], "iB_imn": [108, 108],
        "ident": [128, 128], "ones_bf": [128, 128],
    }
    cst = {nm: nc.declare_dram_parameter(nm, shp, BF16, isOutput=False)
           for nm, shp in cshapes.items()}
    out_e = nc.declare_dram_parameter("out", [BPC, C, L], F32, isOutput=True)

    ymod_d = nc.dram_tensor("ymod", [BPC, C, N], BF16)
    yconv_d = nc.dram_tensor("yconv", [BPC, C, N], BF16)

    MM = nc.tensor.matmul

    with tile.TileContext(nc) as tc, \
         nc.allow_low_precision(reason="bf16 datapath, fp32 psum accumulation"), \
         tc.tile_pool(name="sing", bufs=1) as sing:
        if True:
            zc = sing.tile([128, 1], F32)
            nc.vector.memset(zc, 0.0)
            nc.const_aps.aps[(F32, 0.0)] = zc[:, :]
            ec = sing.tile([128, 1], F32)
            nc.vector.memset(ec, EPS)
            nc.const_aps.aps[(F32, EPS)] = ec[:, :]
            sb = {}
            for qi, nm in enumerate(("fA_re", "fA_im", "fA_imn", "iA_re",
                                     "iA_im", "iA_imn")):
                t = sing.tile([N1, N2, N1], BF16, tag=nm)
                deng = nc.sync if qi % 2 == 0 else nc.gpsimd
                deng.dma_start(out=t, in_=cst[nm].rearrange("a b c -> b a c"))
                sb[nm] = t
            for nm in ("fB_re", "fB_im", "fB_imn", "iB_re", "iB_im", "iB_imn"):
                t = sing.tile([108, 108], BF16, tag=nm)
                nc.sync.dma_start(out=t, in_=cst[nm][:, :])
                sb[nm] = t
            ident = sing.tile([128, 128], BF16)
            nc.sync.dma_start(out=ident, in_=cst["ident"][:, :])
            ident32 = sing.tile([128, 128], F32)
            nc.scalar.activation(ident32, ident, AF.Copy)
            ones_bf = sing.tile([128, 128], BF16)
            nc.sync.dma_start(out=ones_bf, in_=cst["ones_bf"][:, :])
            adawT = sing.tile([128, 18, 128], BF16)
            nc.sync.dma_start(out=adawT,
                              in_=adawT_e.rearrange("k (c o) -> k c o", c=18))
            adab = sing.tile([128, 18], F32)
            nc.sync.dma_start(out=adab, in_=adab_e[:, :])
            w1T = sing.tile([128, CT, C], BF16)
            nc.sync.dma_start(out=w1T,
                              in_=w1T_e.rearrange("(a k) o -> k a o", k=128))
            w2T = sing.tile([128, CT, C], BF16)
            nc.sync.dma_start(out=w2T,
                              in_=w2T_e.rearrange("(a k) o -> k a o", k=128))
            b1c = sing.tile([128, CT], F32)
            nc.sync.dma_start(out=b1c, in_=b1_e[:, :])
            b2c = sing.tile([128, CT], F32)
            nc.sync.dma_start(out=b2c, in_=b2_e[:, :])
            dcol = sing.tile([128, CT], F32)
            nc.sync.dma_start(out=dcol, in_=d_e[:, :])

            import os as _os
            _STAGES = int(_os.environ.get("KSTAGES", "4"))
            # ---------------- phase 1: LN1 + modulate -> ymod -------
            if _STAGES >= 2:
             with tc.tile_pool(name="p1", bufs=3) as p1, \
                 tc.tile_pool(name="p1p", bufs=1, space="PSUM") as p1p, \
                 tc.tile_pool(name="p1q", bufs=3, space="PSUM") as p1q:
                zpadN = p1.tile([128, N - L], BF16, tag="zpadN")
                nc.vector.memset(zpadN, 0.0)
                for b in range(BPC):
                    for ct in range(CT):
                        nc.gpsimd.dma_start(
                            out=ymod_d[b, ct * 128:(ct + 1) * 128, L:N],
                            in_=zpadN)
                for b in range(BPC):
                    for ch in range(NCH):
                        l0 = ch * LCH
                        xf = p1.tile([128, CT, LCH], F32, tag="xf")
                        for ct in range(CT):
                            dq = nc.sync if ct != 1 else nc.scalar
                            dq.dma_start(
                                out=xf[:, ct, :],
                                in_=x_e[b, ct * 128:(ct + 1) * 128, l0:l0 + LCH])
                        xb = p1.tile([128, CT, LCH], BF16, tag="xb")
                        x2 = p1.tile([128, CT, LCH], BF16, tag="x2")
                        for ct in range(CT):
                            if ct == 0:
                                nc.vector.tensor_copy(xb[:, ct, :], xf[:, ct, :])
                            else:
                                nc.scalar.activation(xb[:, ct, :], xf[:, ct, :],
                                                     AF.Copy)
                            nc.gpsimd.tensor_mul(x2[:, ct, :], xb[:, ct, :],
                                                 xb[:, ct, :])
                        s1p = p1p.tile([128, LCH], F32, tag="s1p")
                        s2p = p1p.tile([128, LCH], F32, tag="s2p")
                        for ct in range(CT):
                            MM(s1p, ones_bf, xb[:, ct, :],
                               start=(ct == 0), stop=(ct == CT - 1))
                        for ct in range(CT):
                            MM(s2p, ones_bf, x2[:, ct, :],
                               start=(ct == 0), stop=(ct == CT - 1))
                        mu = p1.tile([128, LCH], BF16, tag="mu")
                        ex2 = p1.tile([128, LCH], F32, tag="ex2")
                        nc.scalar.activation(mu, s1p, AF.Copy, scale=1.0 / C)
                        nc.scalar.activation(ex2, s2p, AF.Copy, scale=1.0 / C)
                        musq = p1.tile([128, LCH], F32, tag="musq")
                        nc.gpsimd.tensor_mul(musq, mu, mu)
                        var = p1.tile([128, LCH], F32, tag="var")
                        nc.vector.tensor_sub(var, ex2, musq)
                        sd_ = p1.tile([128, LCH], F32, tag="sd_")
                        nc.scalar.activation(sd_, var, AF.Sqrt, bias=EPS)
                        inv = p1.tile([128, LCH], BF16, tag="inv")
                        nc.vector.reciprocal(inv, sd_)
                        muinv = p1.tile([128, LCH], BF16, tag="muinv")
                        nc.vector.tensor_mul(muinv, mu, inv)
                        tcf = p1.tile([128, LCH], F32, tag="tcf")
                        nc.sync.dma_start(out=tcf, in_=tc_e[b, :, l0:l0 + LCH])
                        tsil = p1.tile([128, LCH], BF16, tag="tsil")
                        nc.scalar.activation(tsil, tcf, AF.Silu)
                        ym = p1.tile([128, CT, LCH], BF16, tag="ym")
                        for ct in range(CT):
                            adp = p1q.tile([128, 2, LCH], F32, tag="adp")
                            MM(adp[:, 0, :], adawT[:, ct, :], tsil,
                               start=True, stop=True)
                            MM(adp[:, 1, :], adawT[:, 3 + ct, :], tsil,
                               start=True, stop=True)
                            m1 = p1.tile([128, LCH], BF16, tag=f"m1_{ct}")
                            nc.gpsimd.tensor_mul(m1, xb[:, ct, :], inv)
                            z = p1.tile([128, LCH], BF16, tag=f"z_{ct}")
                            nc.gpsimd.tensor_sub(z, m1, muinv)
                            t_ = p1.tile([128, LCH], BF16, tag=f"t_{ct}")
                            nc.vector.scalar_tensor_tensor(
                                t_, adp[:, 1, :], adab[:, 3 + ct:4 + ct],
                                z, op0=AX.add, op1=AX.mult)
                            y1 = p1.tile([128, LCH], BF16, tag=f"y1_{ct}")
                            nc.gpsimd.tensor_add(y1, t_, z)
                            nc.vector.scalar_tensor_tensor(
                                ym[:, ct, :], adp[:, 0, :],
                                adab[:, ct:ct + 1], y1, op0=AX.add, op1=AX.add)
                        for ct in range(CT):
                            nc.gpsimd.dma_start(
                                out=ymod_d[b, ct * 128:(ct + 1) * 128,
                                           l0:l0 + LCH],
                                in_=ym[:, ct, :])

            tc.strict_bb_all_engine_barrier()
            # ---------------- phase 2: kernel FFT + conv FFT --------
            if _STAGES >= 3:
             with tc.tile_pool(name="p2", bufs=1) as p2, \
                 tc.tile_pool(name="p2in", bufs=2) as p2in, \
                 tc.tile_pool(name="p2p", bufs=1, space="PSUM") as p2p:

                H2 = [(0, 20), (20, 16)]    # n2 halves (start, count)
                HS = [(0, 8), (8, 8)]        # subgroup halves

                def pair(h):
                    t = "a" if h == 0 else "b"
                    pre = p2p.tile([128, 1024], F32, tag=f"p{t}_re")
                    pim = p2p.tile([128, 1024], F32, tag=f"p{t}_im")
                    return pre, pim

                def unscr_half(dst, psrc, h, eng):
                    # psrc [128,1024]: 2 banks of 10 slots x 48 -> dst ch-major
                    d3 = dst.rearrange("p (c n) -> p c n", n=N2)
                    st, cnt = H2[h]
                    s5 = psrc.rearrange("p (bk r) -> p bk r", bk=2)[
                        :, :, 0:480].rearrange("p bk (sl c) -> p bk sl c",
                                               sl=10)
                    act = eng is nc.scalar
                    if cnt == 20:
                        o = d3[:, :, st:st + 20].rearrange(
                            "p c (bk sl) -> p bk sl c", bk=2)
                        if act:
                            eng.activation(o, s5[:, :, :, 0:48], AF.Copy)
                        else:
                            eng.tensor_copy(o, s5[:, :, :, 0:48])
                    else:
                        o1 = d3[:, :, st:st + 10].rearrange("p c n -> p n c")
                        o2 = d3[:, :, st + 10:st + 16].rearrange(
                            "p c n -> p n c")
                        if act:
                            eng.activation(o1, s5[:, 0, :, 0:48], AF.Copy)
                            eng.activation(o2, s5[:, 1, 0:6, 0:48], AF.Copy)
                        else:
                            eng.tensor_copy(o1, s5[:, 0, :, 0:48])
                            eng.tensor_copy(o2, s5[:, 1, 0:6, 0:48])

                def f1_half(pre, pim, h, zr, zi, real):
                    st, cnt = H2[h]
                    for jx in range(cnt):
                        n2 = st + jx
                        off = (jx // 10) * 512 + (jx % 10) * 48
                        if real:
                            MM(pre[:, off:off + GS], sb["fA_re"][0:29, n2, :],
                               zr[:, :, n2], start=True, stop=True)
                            MM(pim[:, off:off + GS], sb["fA_im"][0:29, n2, :],
                               zr[:, :, n2], start=True, stop=True)
                        else:
                            MM(pre[:, off:off + GS], sb["fA_re"][:, n2, :],
                               zr[:, :, n2], start=True, stop=False)
                            MM(pim[:, off:off + GS], sb["fA_im"][:, n2, :],
                               zr[:, :, n2], start=True, stop=False)
                            MM(pre[:, off:off + GS], sb["fA_imn"][:, n2, :],
                               zi[:, :, n2], start=False, stop=True)
                            MM(pim[:, off:off + GS], sb["fA_re"][:, n2, :],
                               zi[:, :, n2], start=False, stop=True)

                def i4_half(pre, pim, h, vr3, vi3):
                    st, cnt = H2[h]
                    for jx in range(cnt):
                        n2 = st + jx
                        off = (jx // 10) * 512 + (jx % 10) * 48
                        MM(pre[:, off:off + GS], sb["iA_re"][:, n2, :],
                           vr3[:, :, n2], start=True, stop=False)
                        MM(pim[:, off:off + GS], sb["iA_im"][:, n2, :],
                           vr3[:, :, n2], start=True, stop=False)
                        MM(pre[:, off:off + GS], sb["iA_imn"][:, n2, :],
                           vi3[:, :, n2], start=False, stop=True)
                        MM(pim[:, off:off + GS], sb["iA_re"][:, n2, :],
                           vi3[:, :, n2], start=False, stop=True)

                def t_half(pre, pim, h, inre, inim):
                    st, _ = HS[h]
                    for s in range(st, st + 8):
                        off = ((s - st) // 4) * 512 + ((s - st) % 4) * 128
                        isl = slice(s * 108, (s + 1) * 108)
                        MM(pre[:108, off:off + 128], inre[:, isl], ident32,
                           is_transpose=True, start=True, stop=True)
                        MM(pim[:108, off:off + 128], inim[:, isl], ident32,
                           is_transpose=True, start=True, stop=True)

                def tb_half(pre, pim, h, inre, inim):
                    st, _ = HS[h]
                    for s in range(st, st + 8):
                        off = ((s - st) // 4) * 512 + ((s - st) % 4) * 108
                        isl = slice(s * 128, (s + 1) * 128)
                        MM(pre[:, off:off + 108], inre[:108, isl],
                           ident32[:108, :108], is_transpose=True, start=True,
                           stop=True)
                        MM(pim[:, off:off + 108], inim[:108, isl],
                           ident32[:108, :108], is_transpose=True, start=True,
                           stop=True)

                def d36_half(pre, pim, h, Bre, Bim, Bimn, inre, inim):
                    st, _ = HS[h]
                    for s in range(st, st + 8):
                        off = (s - st) * 128
                        sl = slice(s * 128, (s + 1) * 128)
                        MM(pre[:108, off:off + 128], Bre, inre[:, sl],
                           start=True, stop=False)
                        MM(pim[:108, off:off + 128], Bim, inre[:, sl],
                           start=True, stop=False)
                        MM(pre[:108, off:off + 128], Bimn, inim[:, sl],
                           start=False, stop=True)
                        MM(pim[:108, off:off + 128], Bre, inim[:, sl],
                           start=False, stop=True)

                def hcopy(dst, psrc, h, eng):
                    o = dst[:, h * 1024:(h + 1) * 1024]
                    if eng is nc.scalar:
                        eng.activation(o, psrc[:108, :], AF.Copy)
                    else:
                        eng.tensor_copy(o, psrc[:108, :])

                def vcopy_half(vflat, psrc, h, eng):
                    for bk in range(2):
                        o = vflat[:, (h * 8 + bk * 4) * 108:
                                  (h * 8 + bk * 4) * 108 + 432]
                        s_ = psrc[:, bk * 512:bk * 512 + 432]
                        if eng is nc.scalar:
                            eng.activation(o, s_, AF.Copy)
                        else:
                            eng.tensor_copy(o, s_)

                for g in range(NG):
                    c0 = g * GS
                    # ======== kernel FFT: precomputed host-side ========
                    kh_re = p2in.tile([108, NSUB * 128], BF16, tag="kh_re")
                    kh_im = p2in.tile([108, NSUB * 128], BF16, tag="kh_im")
                    nc.gpsimd.dma_start(out=kh_re, in_=khre_e[g])
                    nc.gpsimd.dma_start(out=kh_im, in_=khim_e[g])

                    # ======== data FFT (batch pair packed complex) ========
                    z_re = p2in.tile([128, GS, N2], BF16, tag="z_re")
                    z_im = p2in.tile([128, GS, N2], BF16, tag="z_im")
                    nc.sync.dma_start(
                        out=z_re, in_=ymod_d[0, c0:c0 + GS, :].rearrange(
                            "c (a b) -> a c b", b=N2))
                    nc.sync.dma_start(
                        out=z_im, in_=ymod_d[1, c0:c0 + GS, :].rearrange(
                            "c (a b) -> a c b", b=N2))
                    S_re = p2.tile([128, GS * N2], F32, tag="S_re")
                    S_im = p2.tile([128, GS * N2], F32, tag="S_im")
                    for h in (0, 1):
                        pre, pim = pair(h)
                        f1_half(pre, pim, h, z_re, z_im, False)
                        unscr_half(S_re, pre, h, nc.scalar)
                        unscr_half(S_im, pim, h, nc.vector)
                    ST_re = p2.tile([108, NSUB * 128], BF16, tag="ST_re")
                    ST_im = p2.tile([108, NSUB * 128], BF16, tag="ST_im")
                    for h in (0, 1):
                        pre, pim = pair(h)
                        t_half(pre, pim, h, S_re, S_im)
                        hcopy(ST_re, pre, h, nc.scalar)
                        hcopy(ST_im, pim, h, nc.vector)
                    X_re = p2.tile([108, NSUB * 128], BF16, tag="X_re")
                    X_im = p2.tile([108, NSUB * 128], BF16, tag="X_im")
                    for h in (0, 1):
                        pre, pim = pair(h)
                        d36_half(pre, pim, h, sb["fB_re"], sb["fB_im"],
                                 sb["fB_imn"], ST_re, ST_im)
                        hcopy(X_re, pre, h, nc.scalar)
                        hcopy(X_im, pim, h, nc.vector)
                    # spectral multiply (per half to keep pipeline fine-grained)
                    Y_re = p2.tile([108, NSUB * 128], BF16, tag="Y_re")
                    Y_im = p2.tile([108, NSUB * 128], BF16, tag="Y_im")
                    q1 = p2.tile([108, NSUB * 128], BF16, tag="q1")
                    q2 = p2.tile([108, NSUB * 128], BF16, tag="q2")
                    for h in (0, 1):
                        sl = slice(h * 1024, (h + 1) * 1024)
                        nc.vector.tensor_mul(q1[:, sl], X_re[:, sl],
                                             kh_re[:, sl])
                        nc.gpsimd.tensor_mul(q2[:, sl], X_im[:, sl],
                                             kh_im[:, sl])
                        nc.gpsimd.tensor_sub(Y_re[:, sl], q1[:, sl],
                                             q2[:, sl])
                        nc.vector.tensor_mul(q1[:, sl], X_re[:, sl],
                                             kh_im[:, sl])
                        nc.gpsimd.tensor_mul(q2[:, sl], X_im[:, sl],
                                             kh_re[:, sl])
                        nc.vector.tensor_add(Y_im[:, sl], q1[:, sl],
                                             q2[:, sl])
                    U_re = p2.tile([108, NSUB * 128], F32, tag="U_re")
                    U_im = p2.tile([108, NSUB * 128], F32, tag="U_im")
                    for h in (0, 1):
                        pre, pim = pair(h)
                        d36_half(pre, pim, h, sb["iB_re"], sb["iB_im"],
                                 sb["iB_imn"], Y_re, Y_im)
                        hcopy(U_re, pre, h, nc.scalar)
                        hcopy(U_im, pim, h, nc.vector)
                    V_re = p2.tile([128, GS, N2], BF16, tag="V_re")
                    V_im = p2.tile([128, GS, N2], BF16, tag="V_im")
                    vr = V_re.rearrange("p a b -> p (a b)")
                    vi = V_im.rearrange("p a b -> p (a b)")
                    for h in (0, 1):
                        pre, pim = pair(h)
                        tb_half(pre, pim, h, U_re, U_im)
                        vcopy_half(vr, pre, h, nc.scalar)
                        vcopy_half(vi, pim, h, nc.vector)
                    yo_re = p2.tile([128, GS, N2], BF16, tag="yo_re")
                    yo_im = p2.tile([128, GS, N2], BF16, tag="yo_im")
                    yof_re = yo_re.rearrange("p a b -> p (a b)")
                    yof_im = yo_im.rearrange("p a b -> p (a b)")
                    for h in (0, 1):
                        pre, pim = pair(h)
                        i4_half(pre, pim, h, V_re, V_im)
                        unscr_half(yof_re, pre, h, nc.scalar)
                        unscr_half(yof_im, pim, h, nc.vector)
                    nc.gpsimd.dma_start(
                        out=yconv_d[0, c0:c0 + GS, :].rearrange(
                            "c (a b) -> a c b", b=N2), in_=yo_re)
                    nc.gpsimd.dma_start(
                        out=yconv_d[1, c0:c0 + GS, :].rearrange(
                            "c (a b) -> a c b", b=N2), in_=yo_im)

            tc.strict_bb_all_engine_barrier()
            # ---------------- phase 3 ------------------------------
            if _STAGES >= 4:
             with tc.tile_pool(name="p3", bufs=1) as p3, \
                 tc.tile_pool(name="p3p", bufs=1, space="PSUM") as p3p, \
                 tc.tile_pool(name="p3pm", bufs=2, space="PSUM") as p3pm:
                for ch in range(NCH):
                    for b in range(BPC):
                        l0 = ch * LCH
                        yc = p3.tile([128, CT, LCH], BF16, tag="yc3" + str(b))
                        ym3 = p3.tile([128, CT, LCH], BF16, tag="ym3" + str(b))
                        xb = p3.tile([128, CT, LCH], BF16, tag="xb3" + str(b))
                        for ct in range(CT):
                            xstg = p3.tile([128, LCH], F32, tag="xstg" + str(b))
                            nc.sync.dma_start(
                                out=xstg,
                                in_=x_e[b, ct * 128:(ct + 1) * 128, l0:l0 + LCH])
                            nc.sync.dma_start(
                                out=yc[:, ct, :],
                                in_=yconv_d[b, ct * 128:(ct + 1) * 128,
                                            SS + l0:SS + l0 + LCH])
                            nc.sync.dma_start(
                                out=ym3[:, ct, :],
                                in_=ymod_d[b, ct * 128:(ct + 1) * 128,
                                           l0:l0 + LCH])
                            nc.scalar.activation(xb[:, ct, :], xstg, AF.Copy)
                        tcf = p3.tile([128, LCH], F32, tag="tcf3" + str(b))
                        nc.sync.dma_start(out=tcf, in_=tc_e[b, :, l0:l0 + LCH])
                        tsil = p3.tile([128, LCH], BF16, tag="tsil3" + str(b))
                        nc.scalar.activation(tsil, tcf, AF.Silu)
                        # gate_tm chunks -> SBUF
                        gts = p3.tile([128, CT, LCH], BF16, tag="gts" + str(b))
                        for ct in range(CT):
                            adp3 = p3pm.tile([128, LCH], F32, tag="adp3")
                            MM(adp3, adawT[:, 6 + ct, :], tsil,
                               start=True, stop=True)
                            nc.vector.tensor_scalar(
                                gts[:, ct, :], adp3, adab[:, 6 + ct:7 + ct],
                                None, AX.add)
                        x1 = p3.tile([128, CT, LCH], BF16, tag="x1" + str(b))
                        x2t = p3.tile([128, CT, LCH], BF16, tag="x2t" + str(b))
                        for ct in range(CT):
                            s1 = p3.tile([128, LCH], BF16, tag=f"s1_{ct}_{b}")
                            nc.vector.scalar_tensor_tensor(
                                s1, ym3[:, ct, :], dcol[:, ct:ct + 1],
                                yc[:, ct, :], op0=AX.mult, op1=AX.add)
                            gt = p3.tile([128, LCH], BF16, tag=f"gt_{ct}_{b}")
                            nc.vector.tensor_mul(gt, gts[:, ct, :], s1)
                            nc.gpsimd.tensor_add(x1[:, ct, :], xb[:, ct, :], gt)
                            nc.gpsimd.tensor_mul(x2t[:, ct, :], x1[:, ct, :],
                                                 x1[:, ct, :])
                        s1p3 = p3p.tile([128, LCH], F32, tag="s1p3" + str(b))
                        s2p3 = p3p.tile([128, LCH], F32, tag="s2p3" + str(b))
                        for ct in range(CT):
                            MM(s1p3, ones_bf, x1[:, ct, :],
                               start=(ct == 0), stop=(ct == CT - 1))
                        for ct in range(CT):
                            MM(s2p3, ones_bf, x2t[:, ct, :],
                               start=(ct == 0), stop=(ct == CT - 1))
                        mu = p3.tile([128, LCH], BF16, tag="mu3" + str(b))
                        ex2 = p3.tile([128, LCH], BF16, tag="ex23" + str(b))
                        nc.scalar.activation(mu, s1p3, AF.Copy,
                                             scale=1.0 / C)
                        nc.scalar.activation(ex2, s2p3, AF.Copy,
                                             scale=1.0 / C)
                        musq = p3.tile([128, LCH], BF16, tag="musq3" + str(b))
                        nc.gpsimd.tensor_mul(musq, mu, mu)
                        var = p3.tile([128, LCH], BF16, tag="var3" + str(b))
                        nc.vector.tensor_sub(var, ex2, musq)
                        sd_ = p3.tile([128, LCH], BF16, tag="sd_3" + str(b))
                        nc.scalar.activation(sd_, var, AF.Sqrt, bias=EPS)
                        inv = p3.tile([128, LCH], BF16, tag="inv3" + str(b))
                        nc.vector.reciprocal(inv, sd_)
                        muinv = p3.tile([128, LCH], BF16, tag="muinv3" + str(b))
                        nc.vector.tensor_mul(muinv, mu, inv)
                        # ada chunks 9-14 -> SBUF
                        cms = p3.tile([128, 6, LCH], BF16, tag="cms" + str(b))
                        for oc in range(6):
                            adp3 = p3pm.tile([128, LCH], F32, tag="adp3")
                            MM(adp3, adawT[:, 9 + oc, :], tsil,
                               start=True, stop=True)
                            nc.scalar.activation(cms[:, oc, :], adp3, AF.Identity,
                                                 bias=adab[:, 9 + oc:10 + oc])
                        z2 = p3.tile([128, CT, LCH], BF16, tag="z2" + str(b))
                        for ct in range(CT):
                            m1 = p3.tile([128, LCH], BF16, tag=f"m13_{ct}_{b}")
                            nc.gpsimd.tensor_mul(m1, x1[:, ct, :], inv)
                            z = p3.tile([128, LCH], BF16, tag=f"z3_{ct}_{b}")
                            nc.gpsimd.tensor_sub(z, m1, muinv)
                            t_ = p3.tile([128, LCH], BF16, tag=f"t3_{ct}_{b}")
                            nc.vector.tensor_mul(t_, cms[:, 3 + ct, :], z)
                            y1 = p3.tile([128, LCH], BF16, tag=f"y13_{ct}_{b}")
                            nc.gpsimd.tensor_add(y1, t_, z)
                            nc.vector.tensor_add(z2[:, ct, :],
                                                 cms[:, ct, :], y1)
                        # MLP layer 1
                        h = p3.tile([128, CT, LCH], BF16, tag="h" + str(b))
                        for oc in range(CT):
                            hp = p3pm.tile([128, LCH], F32, tag="mlp")
                            for ct in range(CT):
                                MM(hp, w1T[:, ct, oc * 128:(oc + 1) * 128],
                                   z2[:, ct, :], start=(ct == 0),
                                   stop=(ct == CT - 1))
                            nc.scalar.activation(h[:, oc, :], hp, AF.Gelu,
                                                 bias=b1c[:, oc:oc + 1])
                        # gate_cm -> SBUF
                        gcs = p3.tile([128, CT, LCH], BF16, tag="gcs" + str(b))
                        for ct in range(CT):
                            adp3 = p3pm.tile([128, LCH], F32, tag="adp3")
                            MM(adp3, adawT[:, 15 + ct, :], tsil,
                               start=True, stop=True)
                            nc.vector.tensor_scalar(
                                gcs[:, ct, :], adp3, adab[:, 15 + ct:16 + ct],
                                None, AX.add)
                        for oc in range(CT):
                            mp = p3pm.tile([128, LCH], F32, tag="mlp")
                            for ct in range(CT):
                                MM(mp, w2T[:, ct, oc * 128:(oc + 1) * 128],
                                   h[:, ct, :], start=(ct == 0),
                                   stop=(ct == CT - 1))
                            mb = p3.tile([128, LCH], BF16, tag=f"mb_{oc}_{b}")
                            nc.scalar.activation(mb, mp, AF.Identity,
                                                 bias=b2c[:, oc:oc + 1])
                            gc = p3.tile([128, LCH], BF16, tag=f"gc_{oc}_{b}")
                            nc.vector.tensor_mul(gc, gcs[:, oc, :], mb)
                            ostg = p3.tile([128, LCH], F32, tag="ostg" + str(b))
                            nc.vector.tensor_add(ostg, x1[:, oc, :], gc)
                            nc.gpsimd.dma_start(
                                out=out_e[b, oc * 128:(oc + 1) * 128,
                                          l0:l0 + LCH],
                                in_=ostg)
    nc.finalize()
    return nc


def kernel(x, t_cond, kernels, D, ada_w, ada_b, w1, b1, w2, b2):
    global _last_results
    consts = _make_consts()
    khre, khim = _make_khat(kernels)
    nc = build_graph()
    shared = {
        "khre": khre,
        "khim": khim,
        "DT": np.ascontiguousarray(
            np.asarray(D, np.float32).reshape(CT, 128).T),
        "ada_wT": np.ascontiguousarray(ada_w.T).astype(ml_dtypes.bfloat16),
        "ada_bT": np.ascontiguousarray(
            np.asarray(ada_b, np.float32).reshape(18, 128).T),
        "w1T": np.ascontiguousarray(w1.T).astype(ml_dtypes.bfloat16),
        "b1T": np.ascontiguousarray(
            np.asarray(b1, np.float32).reshape(CT, 128).T),
        "w2T": np.ascontiguousarray(w2.T).astype(ml_dtypes.bfloat16),
        "b2T": np.ascontiguousarray(
            np.asarray(b2, np.float32).reshape(CT, 128).T),
    }
    shared.update(consts)
    in_maps = []
    for i in range(NCORES):
        m = dict(shared)
        m["x"] = np.ascontiguousarray(x[i * BPC:(i + 1) * BPC], dtype=np.float32)
        m["t_cond"] = np.ascontiguousarray(t_cond[i * BPC:(i + 1) * BPC],
                                           dtype=np.float32)
        in_maps.append(m)
    trace = os.environ.get("KERNEL_TRACE", "0") == "1"
    res = run_bass_kernel_spmd(nc, in_maps, list(range(NCORES)), trace=trace)
    _last_results = res
    outs = [r["out"] if isinstance(r, dict) else r for r in res.results]
    return np.concatenate([np.asarray(o, dtype=np.float32).reshape(BPC, C, L)
                           for o in outs], axis=0)


if __name__ == "__main__":
    build_graph()
    print("graph built ok")



# revision 35
# speedup vs baseline: 1.2873x; 1.0984x over previous
"""AdaConvBlock Trainium2 kernel: 8-core data-parallel (2 batch elems/core).

Per core (b=2, C=384, L=4096):
  LN1 -> adaLN modulate -> SLConv (1024-tap depthwise conv via four-step
  matmul FFT, N=4608=128x36, batch pair packed as complex) + D-skip ->
  gated residual -> LN2 -> modulate -> pointwise MLP (gelu) -> gated residual.

FFT: Cooley-Tukey twiddles folded into 36 per-n2 stationary matrices ->
pure matmul FFT, no twiddle pointwise work. PSUM is managed in explicit
2KB-bank slots (one matmul output per slot, has_written gives per-slot
overwrite/accumulate semantics).
"""
import os
import sys

sys.path.insert(0, "/opt/trn_rl_repo")

import numpy as np
import ml_dtypes

import concourse.bass as bass
import concourse.bacc as bacc
import concourse.tile as tile
from concourse import mybir
from concourse.bass_utils import run_bass_kernel_spmd

F32 = mybir.dt.float32
BF16 = mybir.dt.bfloat16
AX = mybir.AluOpType
AF = mybir.ActivationFunctionType

B, C, L = 16, 384, 4096
NCORES = 8
BPC = B // NCORES
CT = 3
NCH = 8
LCH = 512
N, N1, N2 = 4608, 128, 36
KLEN = 1024
SS = 512
NG = 8
GS = 48
NSUB = 16
EPS = 1e-5
DECAY = 2.0
NS, KS = 6, 32

_last_results = None


def _make_consts():
    k1 = np.arange(N1)
    n1 = np.arange(N1)
    W128 = np.exp(-2j * np.pi * np.outer(k1, n1) / N1)
    W36 = np.exp(-2j * np.pi * np.outer(np.arange(N2), np.arange(N2)) / N2)
    fA = np.zeros((N2, N1, N1), complex)
    iA = np.zeros((N2, N1, N1), complex)
    for n2 in range(N2):
        M = np.exp(-2j * np.pi * n2 * k1 / N)[:, None] * W128      # [k1,n1]
        fA[n2] = M.T                                               # lhsT [n1,k1]
        IA = (np.exp(2j * np.pi * n2 * k1 / N)[None, :] * np.conj(W128).T) / N
        iA[n2] = IA.T                                              # lhsT [k1,n1]

    def blockdiag3(Mx):
        out = np.zeros((108, 108), complex)
        for s in range(3):
            out[s * 36:(s + 1) * 36, s * 36:(s + 1) * 36] = Mx
        return out

    fB = blockdiag3(W36.T)
    iB = blockdiag3(np.conj(W36).T)

    def bf(x):
        return np.ascontiguousarray(x).astype(ml_dtypes.bfloat16)

    return {
        "fA_re": bf(fA.real), "fA_im": bf(fA.imag), "fA_imn": bf(-fA.imag),
        "iA_re": bf(iA.real), "iA_im": bf(iA.imag), "iA_imn": bf(-iA.imag),
        "fB_re": bf(fB.real), "fB_im": bf(fB.imag), "fB_imn": bf(-fB.imag),
        "iB_re": bf(iB.real), "iB_im": bf(iB.imag), "iB_imn": bf(-iB.imag),
        "ident": bf(np.eye(128)),
        "ones_bf": bf(np.ones((128, 128))),
        "ones_c": bf(np.full((128, 128), 1.0 / C)),
        "ones_row": bf(np.ones((1, 1024))),
    }


def _make_khat(kernels):
    """Host-side kernel FFT: build the normalized multi-scale kernel, FFT to
    length N=4608, and lay out per group in the spectral-domain tile layout
    produced by the on-device fwd FFT: KH[g, c_loc*36+k2, s*128+k1] =
    FFT(k_c)[k1 + 128*k2] with c = 48g + 3s + c_loc."""
    ker = np.asarray(kernels, np.float64)  # (NS, 1, C, KS)
    klist = [np.repeat(ker[i, 0], 2 ** max(0, i - 1), axis=-1)
             * (DECAY ** (NS - i - 1)) for i in range(NS)]
    k = np.concatenate(klist, axis=-1)  # (C, KLEN)
    k = k / np.linalg.norm(k, axis=-1, keepdims=True)
    Ksp = np.fft.fft(k, N, axis=-1)  # (C, N)
    Kp = Ksp.reshape(C, N2, N1)                      # [c, k2, k1]
    Kp = Kp.reshape(NG, GS // 3, 3, N2, N1)          # [g, s, c_loc, k2, k1]
    Kp = Kp.transpose(0, 2, 3, 1, 4).reshape(NG, 108, NSUB * 128)

    def bf(x):
        return np.ascontiguousarray(x).astype(ml_dtypes.bfloat16)

    return bf(Kp.real), bf(Kp.imag)


# bank-slot offset for per-n2 FFT matmul outputs: 10 slots of 48 per 2KB bank
def _n2off(n2):
    return (n2 // 10) * 512 + (n2 % 10) * 48


def _fold_ada_bias(ada_b):
    """[128, 18] bias columns; scale chunks (3..5 tm, 12..14 cm) get +1 so
    modulate is y = z*scale' + shift with scale' = 1 + scale."""
    ab = np.asarray(ada_b, np.float32).reshape(18, 128).T.copy()
    ab[:, 3:6] += 1.0
    ab[:, 12:15] += 1.0
    return ab


def build_graph():
    nc = bacc.Bacc(None)

    x_e = nc.declare_dram_parameter("x", [BPC, C, L], F32, isOutput=False)
    tc_e = nc.declare_dram_parameter("t_cond", [BPC, C // 3, L], F32, isOutput=False)
    khre_e = nc.declare_dram_parameter("khre", [NG, 108, NSUB * 128], BF16,
                                       isOutput=False)
    khim_e = nc.declare_dram_parameter("khim", [NG, 108, NSUB * 128], BF16,
                                       isOutput=False)
    d_e = nc.declare_dram_parameter("DT", [128, CT], F32, isOutput=False)
    adawT_e = nc.declare_dram_parameter("ada_wT", [C // 3, 6 * C], BF16, isOutput=False)
    adab_e = nc.declare_dram_parameter("ada_bT", [128, 18], F32, isOutput=False)
    adabR_e = nc.declare_dram_parameter("ada_bR", [1, 18 * 128], BF16,
                                        isOutput=False)
    w1T_e = nc.declare_dram_parameter("w1T", [C, C], BF16, isOutput=False)
    b1_e = nc.declare_dram_parameter("b1T", [128, CT], F32, isOutput=False)
    w2T_e = nc.declare_dram_parameter("w2T", [C, C], BF16, isOutput=False)
    b2_e = nc.declare_dram_parameter("b2T", [128, CT], F32, isOutput=False)
    cshapes = {
        "fA_re": [N2, N1, N1], "fA_im": [N2, N1, N1], "fA_imn": [N2, N1, N1],
        "iA_re": [N2, N1, N1], "iA_im": [N2, N1, N1], "iA_imn": [N2, N1, N1],
        "fB_re": [108, 108], "fB_im": [108, 108], "fB_imn": [108, 108],
        "iB_re": [108, 108], "iB_im": [108, 108], "iB_imn": [108, 108],
        "ident": [128, 128], "ones_bf": [128, 128],
        "ones_c": [128, 128], "ones_row": [1, 1024],
    }
    cst = {nm: nc.declare_dram_parameter(nm, shp, BF16, isOutput=False)
           for nm, shp in cshapes.items()}
    out_e = nc.declare_dram_parameter("out", [BPC, C, L], BF16, isOutput=True)

    ymod_d = nc.dram_tensor("ymod", [BPC, C, N], BF16)
    yconv_d = nc.dram_tensor("yconv", [BPC, C, N], BF16)
    x1_d = nc.dram_tensor("x1s", [BPC, C, L], BF16)

    MM = nc.tensor.matmul

    with tile.TileContext(nc) as tc, \
         nc.allow_low_precision(reason="bf16 datapath, fp32 psum accumulation"), \
         tc.tile_pool(name="sing", bufs=1) as sing:
        if True:
            zc = sing.tile([128, 1], F32)
            nc.vector.memset(zc, 0.0)
            nc.const_aps.aps[(F32, 0.0)] = zc[:, :]
            ec = sing.tile([128, 1], F32)
            nc.vector.memset(ec, EPS)
            nc.const_aps.aps[(F32, EPS)] = ec[:, :]
            sb = {}
            qengs = (nc.sync, nc.gpsimd, nc.scalar)
            for qi, nm in enumerate(("fA_re", "fA_im", "fA_imn", "iA_re",
                                     "iA_im", "iA_imn")):
                t = sing.tile([N1, N2, N1], BF16, tag=nm)
                qengs[qi % 3].dma_start(out=t,
                                        in_=cst[nm].rearrange("a b c -> b a c"))
                sb[nm] = t
            for nm in ("fB_re", "fB_im", "fB_imn", "iB_re", "iB_im", "iB_imn"):
                t = sing.tile([108, 108], BF16, tag=nm)
                nc.sync.dma_start(out=t, in_=cst[nm][:, :])
                sb[nm] = t
            ident = sing.tile([128, 128], BF16)
            nc.sync.dma_start(out=ident, in_=cst["ident"][:, :])
            ident32 = sing.tile([128, 128], F32)
            nc.scalar.activation(ident32, ident, AF.Copy)
            ones_bf = sing.tile([128, 128], BF16)
            nc.sync.dma_start(out=ones_bf, in_=cst["ones_bf"][:, :])
            ones_c = sing.tile([128, 128], BF16)
            nc.gpsimd.dma_start(out=ones_c, in_=cst["ones_c"][:, :])
            ones_row = sing.tile([1, 1024], BF16)
            nc.gpsimd.dma_start(out=ones_row, in_=cst["ones_row"][:, :])
            adabR = sing.tile([1, 18, 128], BF16)
            nc.gpsimd.dma_start(out=adabR,
                                in_=adabR_e.rearrange("a (c o) -> a c o", c=18))

            def bias_mm(psl, ch, lch):
                # K=1 matmul: adds ada bias column for chunk ch across lch
                MM(psl, adabR[0:1, ch, :], ones_row[0:1, 0:lch],
                   start=True, stop=False)
            adawT = sing.tile([128, 18, 128], BF16)
            nc.sync.dma_start(out=adawT,
                              in_=adawT_e.rearrange("k (c o) -> k c o", c=18))
            adab = sing.tile([128, 18], F32)
            nc.sync.dma_start(out=adab, in_=adab_e[:, :])
            w1T = sing.tile([128, CT, C], BF16)
            nc.sync.dma_start(out=w1T,
                              in_=w1T_e.rearrange("(a k) o -> k a o", k=128))
            w2T = sing.tile([128, CT, C], BF16)
            nc.sync.dma_start(out=w2T,
                              in_=w2T_e.rearrange("(a k) o -> k a o", k=128))
            b1c = sing.tile([128, CT], F32)
            nc.sync.dma_start(out=b1c, in_=b1_e[:, :])
            b2c = sing.tile([128, CT], F32)
            nc.sync.dma_start(out=b2c, in_=b2_e[:, :])
            dcol = sing.tile([128, CT], F32)
            nc.sync.dma_start(out=dcol, in_=d_e[:, :])

            # silu(t_cond) computed once, resident for ph1 (ada) and ph3
            tsl = sing.tile([128, BPC, L], BF16, tag="tsl")
            with tc.tile_pool(name="p0", bufs=2) as p0:
                for b in range(BPC):
                    tcf = p0.tile([128, L], F32, tag="tcf")
                    nc.sync.dma_start(out=tcf, in_=tc_e[b, :, :])
                    nc.scalar.activation(tsl[:, b, :], tcf, AF.Silu)

            import os as _os
            _STAGES = int(_os.environ.get("KSTAGES", "4"))
            # ---------------- phase 1: LN1 + modulate -> ymod -------
            # NOTE: ada_bT scale chunks (3..5, 12..14) carry a host-folded +1
            # so modulate is y = z*(scale') + shift with scale' = 1+scale.
            if _STAGES >= 2:
             with tc.tile_pool(name="p1", bufs=3) as p1, \
                 tc.tile_pool(name="p1p", bufs=1, space="PSUM") as p1p, \
                 tc.tile_pool(name="p1q", bufs=3, space="PSUM") as p1q:
                zpadN = p1.tile([128, N - L], BF16, tag="zpadN")
                nc.vector.memset(zpadN, 0.0)
                for b in range(BPC):
                    for ct in range(CT):
                        nc.gpsimd.dma_start(
                            out=ymod_d[b, ct * 128:(ct + 1) * 128, L:N],
                            in_=zpadN)
                for b in range(BPC):
                    for ch in range(NCH):
                        l0 = ch * LCH
                        xf = p1.tile([128, CT, LCH], F32, tag="xf")
                        for ct in range(CT):
                            dq = nc.sync if ct != 1 else nc.gpsimd
                            dq.dma_start(
                                out=xf[:, ct, :],
                                in_=x_e[b, ct * 128:(ct + 1) * 128, l0:l0 + LCH])
                        xb = p1.tile([128, CT, LCH], BF16, tag="xb")
                        x2 = p1.tile([128, CT, LCH], BF16, tag="x2")
                        for ct in range(CT):
                            if ct == 2:
                                nc.vector.tensor_copy(xb[:, ct, :], xf[:, ct, :])
                            else:
                                nc.scalar.activation(xb[:, ct, :], xf[:, ct, :],
                                                     AF.Copy)
                            eng = nc.vector if ct != 0 else nc.gpsimd
                            eng.tensor_mul(x2[:, ct, :], xb[:, ct, :],
                                           xb[:, ct, :])
                        sst = p1p.tile([128, 2, LCH], F32, tag="sst")
                        for ct in range(CT):
                            MM(sst[:, 0, :], ones_c, xb[:, ct, :],
                               start=(ct == 0), stop=(ct == CT - 1))
                        for ct in range(CT):
                            MM(sst[:, 1, :], ones_c, x2[:, ct, :],
                               start=(ct == 0), stop=(ct == CT - 1))
                        muex = p1.tile([128, 2, LCH], BF16, tag="muex")
                        nc.scalar.activation(muex, sst, AF.Copy)
                        mu, ex = muex[:, 0, :], muex[:, 1, :]
                        musq = p1.tile([128, LCH], BF16, tag="musq")
                        nc.gpsimd.tensor_mul(musq, mu, mu)
                        var = p1.tile([128, LCH], BF16, tag="var")
                        nc.vector.tensor_sub(var, ex, musq)
                        sd_ = p1.tile([128, LCH], F32, tag="sd_")
                        nc.scalar.activation(sd_, var, AF.Sqrt, bias=EPS)
                        inv = p1.tile([128, LCH], BF16, tag="inv")
                        nc.vector.reciprocal(inv, sd_)
                        muinv = p1.tile([128, LCH], BF16, tag="muinv")
                        nc.vector.tensor_mul(muinv, mu, inv)
                        ym = p1.tile([128, CT, LCH], BF16, tag="ym")
                        for ct in range(CT):
                            adp = p1q.tile([128, 2, LCH], F32, tag="adp")
                            bias_mm(adp[:, 0, :], ct, LCH)
                            MM(adp[:, 0, :], adawT[:, ct, :],
                               tsl[:, b, l0:l0 + LCH], start=False, stop=True)
                            bias_mm(adp[:, 1, :], 3 + ct, LCH)
                            MM(adp[:, 1, :], adawT[:, 3 + ct, :],
                               tsl[:, b, l0:l0 + LCH], start=False, stop=True)
                            m1 = p1.tile([128, LCH], BF16, tag=f"m1_{ct}")
                            eng = nc.vector if ct != 1 else nc.gpsimd
                            eng.tensor_mul(m1, xb[:, ct, :], inv)
                            z = p1.tile([128, LCH], BF16, tag=f"z_{ct}")
                            eng2 = nc.gpsimd if ct != 1 else nc.vector
                            eng2.tensor_sub(z, m1, muinv)
                            u = p1.tile([128, LCH], BF16, tag=f"u_{ct}")
                            if ct != 1:
                                # drain biased PSUM via Act copy; TT at 2x
                                adb = p1.tile([128, 2, LCH], BF16,
                                              tag=f"adb_{ct}")
                                nc.scalar.activation(adb, adp, AF.Copy)
                                srcS, srcSH = adb[:, 1, :], adb[:, 0, :]
                            else:
                                srcS, srcSH = adp[:, 1, :], adp[:, 0, :]
                            nc.vector.tensor_mul(u, srcS, z)
                            nc.vector.tensor_add(ym[:, ct, :], srcSH, u)
                        for ct in range(CT):
                            nc.gpsimd.dma_start(
                                out=ymod_d[b, ct * 128:(ct + 1) * 128,
                                           l0:l0 + LCH],
                                in_=ym[:, ct, :])

            tc.strict_bb_all_engine_barrier()
            # ---------------- phase 2: kernel FFT + conv FFT --------
            if _STAGES >= 3:
             with tc.tile_pool(name="p2", bufs=1) as p2, \
                 tc.tile_pool(name="p2in", bufs=2) as p2in, \
                 tc.tile_pool(name="p2p", bufs=1, space="PSUM") as p2p:

                H2 = [(0, 20), (20, 16)]    # n2 halves (start, count)
                HS = [(0, 8), (8, 8)]        # subgroup halves

                def pair(h):
                    t = "a" if h == 0 else "b"
                    pre = p2p.tile([128, 1024], F32, tag=f"p{t}_re")
                    pim = p2p.tile([128, 1024], F32, tag=f"p{t}_im")
                    return pre, pim

                def unscr_half(dst, psrc, h, eng):
                    # psrc [128,1024]: 2 banks of 10 slots x 48 -> dst ch-major
                    d3 = dst.rearrange("p (c n) -> p c n", n=N2)
                    st, cnt = H2[h]
                    s5 = psrc.rearrange("p (bk r) -> p bk r", bk=2)[
                        :, :, 0:480].rearrange("p bk (sl c) -> p bk sl c",
                                               sl=10)
                    act = eng is nc.scalar
                    if cnt == 20:
                        o = d3[:, :, st:st + 20].rearrange(
                            "p c (bk sl) -> p bk sl c", bk=2)
                        if act:
                            eng.activation(o, s5[:, :, :, 0:48], AF.Copy)
                        else:
                            eng.tensor_copy(o, s5[:, :, :, 0:48])
                    else:
                        o1 = d3[:, :, st:st + 10].rearrange("p c n -> p n c")
                        o2 = d3[:, :, st + 10:st + 16].rearrange(
                            "p c n -> p n c")
                        if act:
                            eng.activation(o1, s5[:, 0, :, 0:48], AF.Copy)
                            eng.activation(o2, s5[:, 1, 0:6, 0:48], AF.Copy)
                        else:
                            eng.tensor_copy(o1, s5[:, 0, :, 0:48])
                            eng.tensor_copy(o2, s5[:, 1, 0:6, 0:48])

                def f1_half(pre, pim, h, zr, zi, real):
                    st, cnt = H2[h]
                    for jx in range(cnt):
                        n2 = st + jx
                        off = (jx // 10) * 512 + (jx % 10) * 48
                        if real:
                            MM(pre[:, off:off + GS], sb["fA_re"][0:29, n2, :],
                               zr[:, :, n2], start=True, stop=True)
                            MM(pim[:, off:off + GS], sb["fA_im"][0:29, n2, :],
                               zr[:, :, n2], start=True, stop=True)
                        else:
                            MM(pre[:, off:off + GS], sb["fA_re"][:, n2, :],
                               zr[:, :, n2], start=True, stop=False)
                            MM(pim[:, off:off + GS], sb["fA_im"][:, n2, :],
                               zr[:, :, n2], start=True, stop=False)
                            MM(pre[:, off:off + GS], sb["fA_imn"][:, n2, :],
                               zi[:, :, n2], start=False, stop=True)
                            MM(pim[:, off:off + GS], sb["fA_re"][:, n2, :],
                               zi[:, :, n2], start=False, stop=True)

                def i4_half(pre, pim, h, vr3, vi3):
                    st, cnt = H2[h]
                    for jx in range(cnt):
                        n2 = st + jx
                        off = (jx // 10) * 512 + (jx % 10) * 48
                        MM(pre[:, off:off + GS], sb["iA_re"][:, n2, :],
                           vr3[:, :, n2], start=True, stop=False)
                        MM(pim[:, off:off + GS], sb["iA_im"][:, n2, :],
                           vr3[:, :, n2], start=True, stop=False)
                        MM(pre[:, off:off + GS], sb["iA_imn"][:, n2, :],
                           vi3[:, :, n2], start=False, stop=True)
                        MM(pim[:, off:off + GS], sb["iA_re"][:, n2, :],
                           vi3[:, :, n2], start=False, stop=True)

                def t_half(pre, pim, h, inre, inim):
                    st, _ = HS[h]
                    for s in range(st, st + 8):
                        off = ((s - st) // 4) * 512 + ((s - st) % 4) * 128
                        isl = slice(s * 108, (s + 1) * 108)
                        MM(pre[:108, off:off + 128], inre[:, isl], ident32,
                           is_transpose=True, start=True, stop=True)
                        MM(pim[:108, off:off + 128], inim[:, isl], ident32,
                           is_transpose=True, start=True, stop=True)

                def tb_half(pre, pim, h, inre, inim):
                    st, _ = HS[h]
                    for s in range(st, st + 8):
                        off = ((s - st) // 4) * 512 + ((s - st) % 4) * 108
                        isl = slice(s * 128, (s + 1) * 128)
                        MM(pre[:, off:off + 108], inre[:108, isl],
                           ident32[:108, :108], is_transpose=True, start=True,
                           stop=True)
                        MM(pim[:, off:off + 108], inim[:108, isl],
                           ident32[:108, :108], is_transpose=True, start=True,
                           stop=True)

                def d36_half(pre, pim, h, Bre, Bim, Bimn, inre, inim):
                    st, _ = HS[h]
                    for s in range(st, st + 8):
                        off = (s - st) * 128
                        sl = slice(s * 128, (s + 1) * 128)
                        MM(pre[:108, off:off + 128], Bre, inre[:, sl],
                           start=True, stop=False)
                        MM(pim[:108, off:off + 128], Bim, inre[:, sl],
                           start=True, stop=False)
                        MM(pre[:108, off:off + 128], Bimn, inim[:, sl],
                           start=False, stop=True)
                        MM(pim[:108, off:off + 128], Bre, inim[:, sl],
                           start=False, stop=True)

                def hcopy(dst, psrc, h, eng):
                    o = dst[:, h * 1024:(h + 1) * 1024]
                    if eng is nc.scalar:
                        eng.activation(o, psrc[:108, :], AF.Copy)
                    else:
                        eng.tensor_copy(o, psrc[:108, :])

                def vcopy_half(vflat, psrc, h, eng):
                    for bk in range(2):
                        o = vflat[:, (h * 8 + bk * 4) * 108:
                                  (h * 8 + bk * 4) * 108 + 432]
                        s_ = psrc[:, bk * 512:bk * 512 + 432]
                        if eng is nc.scalar:
                            eng.activation(o, s_, AF.Copy)
                        else:
                            eng.tensor_copy(o, s_)

                for g in range(NG):
                    c0 = g * GS
                    # ======== kernel FFT: precomputed host-side ========
                    kh_re = p2in.tile([108, NSUB * 128], BF16, tag="kh_re")
                    kh_im = p2in.tile([108, NSUB * 128], BF16, tag="kh_im")
                    nc.gpsimd.dma_start(out=kh_re, in_=khre_e[g])
                    nc.gpsimd.dma_start(out=kh_im, in_=khim_e[g])

                    # ======== data FFT (batch pair packed complex) ========
                    z_re = p2in.tile([128, GS, N2], BF16, tag="z_re")
                    z_im = p2in.tile([128, GS, N2], BF16, tag="z_im")
                    nc.sync.dma_start(
                        out=z_re, in_=ymod_d[0, c0:c0 + GS, :].rearrange(
                            "c (a b) -> a c b", b=N2))
                    nc.sync.dma_start(
                        out=z_im, in_=ymod_d[1, c0:c0 + GS, :].rearrange(
                            "c (a b) -> a c b", b=N2))
                    S_re = p2.tile([128, GS * N2], F32, tag="S_re")
                    S_im = p2.tile([128, GS * N2], F32, tag="S_im")
                    for h in (0, 1):
                        pre, pim = pair(h)
                        f1_half(pre, pim, h, z_re, z_im, False)
                        unscr_half(S_re, pre, h, nc.scalar)
                        unscr_half(S_im, pim, h, nc.vector)
                    ST_re = p2.tile([108, NSUB * 128], BF16, tag="ST_re")
                    ST_im = p2.tile([108, NSUB * 128], BF16, tag="ST_im")
                    for h in (0, 1):
                        pre, pim = pair(h)
                        t_half(pre, pim, h, S_re, S_im)
                        hcopy(ST_re, pre, h, nc.scalar)
                        hcopy(ST_im, pim, h, nc.vector)
                    X_re = p2.tile([108, NSUB * 128], BF16, tag="X_re")
                    X_im = p2.tile([108, NSUB * 128], BF16, tag="X_im")
                    for h in (0, 1):
                        pre, pim = pair(h)
                        d36_half(pre, pim, h, sb["fB_re"], sb["fB_im"],
                                 sb["fB_imn"], ST_re, ST_im)
                        hcopy(X_re, pre, h, nc.scalar)
                        hcopy(X_im, pim, h, nc.vector)
                    # spectral multiply (per half to keep pipeline fine-grained)
                    Y_re = p2.tile([108, NSUB * 128], BF16, tag="Y_re")
                    Y_im = p2.tile([108, NSUB * 128], BF16, tag="Y_im")
                    q1 = p2.tile([108, NSUB * 128], BF16, tag="q1")
                    q2 = p2.tile([108, NSUB * 128], BF16, tag="q2")
                    for h in (0, 1):
                        sl = slice(h * 1024, (h + 1) * 1024)
                        nc.vector.tensor_mul(q1[:, sl], X_re[:, sl],
                                             kh_re[:, sl])
                        nc.gpsimd.tensor_mul(q2[:, sl], X_im[:, sl],
                                             kh_im[:, sl])
                        nc.gpsimd.tensor_sub(Y_re[:, sl], q1[:, sl],
                                             q2[:, sl])
                        nc.vector.tensor_mul(q1[:, sl], X_re[:, sl],
                                             kh_im[:, sl])
                        nc.gpsimd.tensor_mul(q2[:, sl], X_im[:, sl],
                                             kh_re[:, sl])
                        nc.vector.tensor_add(Y_im[:, sl], q1[:, sl],
                                             q2[:, sl])
                    U_re = p2.tile([108, NSUB * 128], F32, tag="U_re")
                    U_im = p2.tile([108, NSUB * 128], F32, tag="U_im")
                    for h in (0, 1):
                        pre, pim = pair(h)
                        d36_half(pre, pim, h, sb["iB_re"], sb["iB_im"],
                                 sb["iB_imn"], Y_re, Y_im)
                        hcopy(U_re, pre, h, nc.scalar)
                        hcopy(U_im, pim, h, nc.vector)
                    V_re = p2.tile([128, GS, N2], BF16, tag="V_re")
                    V_im = p2.tile([128, GS, N2], BF16, tag="V_im")
                    vr = V_re.rearrange("p a b -> p (a b)")
                    vi = V_im.rearrange("p a b -> p (a b)")
                    for h in (0, 1):
                        pre, pim = pair(h)
                        tb_half(pre, pim, h, U_re, U_im)
                        vcopy_half(vr, pre, h, nc.scalar)
                        vcopy_half(vi, pim, h, nc.vector)
                    yo_re = p2.tile([128, GS, N2], BF16, tag="yo_re")
                    yo_im = p2.tile([128, GS, N2], BF16, tag="yo_im")
                    yof_re = yo_re.rearrange("p a b -> p (a b)")
                    yof_im = yo_im.rearrange("p a b -> p (a b)")
                    for h in (0, 1):
                        pre, pim = pair(h)
                        i4_half(pre, pim, h, V_re, V_im)
                        unscr_half(yof_re, pre, h, nc.scalar)
                        unscr_half(yof_im, pim, h, nc.vector)
                    nc.gpsimd.dma_start(
                        out=yconv_d[0, c0:c0 + GS, :].rearrange(
                            "c (a b) -> a c b", b=N2), in_=yo_re)
                    nc.gpsimd.dma_start(
                        out=yconv_d[1, c0:c0 + GS, :].rearrange(
                            "c (a b) -> a c b", b=N2), in_=yo_im)

            tc.strict_bb_all_engine_barrier()
            # ------- phase 3a: residual + gate_tm + LN2 stats (Rsqrt) -------
            # inv2/muinv2 kept SBUF-resident for ph3b; x1 staged via DRAM.
            if _STAGES >= 4:
             with tc.tile_pool(name="p3r", bufs=1) as p3r:
              inv2r = p3r.tile([128, BPC, L], BF16, tag="inv2r")
              muinv2r = p3r.tile([128, BPC, L], BF16, tag="muinv2r")
              with tc.tile_pool(name="p3a", bufs=2) as p3, \
                  tc.tile_pool(name="p3ap", bufs=1, space="PSUM") as p3p, \
                  tc.tile_pool(name="p3aq", bufs=3, space="PSUM") as p3pm:
                for ch in range(NCH):
                    for b in range(BPC):
                        l0 = ch * LCH
                        yc = p3.tile([128, CT, LCH], BF16, tag="yc3")
                        ym3 = p3.tile([128, CT, LCH], BF16, tag="ym3")
                        xf3 = p3.tile([128, CT, LCH], F32, tag="xf3")
                        for ct in range(CT):
                            nc.sync.dma_start(
                                out=xf3[:, ct, :],
                                in_=x_e[b, ct * 128:(ct + 1) * 128, l0:l0 + LCH])
                            nc.sync.dma_start(
                                out=yc[:, ct, :],
                                in_=yconv_d[b, ct * 128:(ct + 1) * 128,
                                            SS + l0:SS + l0 + LCH])
                            nc.gpsimd.dma_start(
                                out=ym3[:, ct, :],
                                in_=ymod_d[b, ct * 128:(ct + 1) * 128,
                                           l0:l0 + LCH])
                        x1 = p3.tile([128, CT, LCH], BF16, tag="x1")
                        x2t = p3.tile([128, CT, LCH], BF16, tag="x2t")
                        for ct in range(CT):
                            adp3 = p3pm.tile([128, LCH], F32, tag="adp3")
                            bias_mm(adp3, 6 + ct, LCH)
                            MM(adp3, adawT[:, 6 + ct, :],
                               tsl[:, b, l0:l0 + LCH], start=False, stop=True)
                            # s1 = D*ym + yconv: tensor_scalar (4x) + TT (2x)
                            dm = p3.tile([128, LCH], BF16, tag=f"dm_{ct}")
                            nc.vector.tensor_scalar(
                                dm, ym3[:, ct, :], dcol[:, ct:ct + 1],
                                None, AX.mult)
                            s1 = p3.tile([128, LCH], BF16, tag=f"s1_{ct}")
                            eng0 = nc.gpsimd if ct == 1 else nc.vector
                            eng0.tensor_add(s1, dm, yc[:, ct, :])
                            # gx = gate_tm' * s1 (bias folded into PSUM)
                            gx = p3.tile([128, LCH], BF16, tag=f"gx_{ct}")
                            if ct != 1:
                                gtb = p3.tile([128, LCH], BF16, tag=f"gtb_{ct}")
                                nc.scalar.activation(gtb, adp3, AF.Copy)
                                nc.vector.tensor_mul(gx, gtb, s1)
                            else:
                                nc.vector.tensor_mul(gx, adp3, s1)
                            eng = nc.vector if ct == 1 else nc.gpsimd
                            eng.tensor_add(x1[:, ct, :], xf3[:, ct, :], gx)
                            nc.scalar.activation(x2t[:, ct, :], x1[:, ct, :],
                                                 AF.Square)
                            nc.gpsimd.dma_start(
                                out=x1_d[b, ct * 128:(ct + 1) * 128,
                                         l0:l0 + LCH],
                                in_=x1[:, ct, :])
                        sst3 = p3p.tile([128, 2, LCH], F32, tag="sst3")
                        for ct in range(CT):
                            MM(sst3[:, 0, :], ones_c, x1[:, ct, :],
                               start=(ct == 0), stop=(ct == CT - 1))
                        for ct in range(CT):
                            MM(sst3[:, 1, :], ones_c, x2t[:, ct, :],
                               start=(ct == 0), stop=(ct == CT - 1))
                        muex3 = p3.tile([128, 2, LCH], BF16, tag="muex3")
                        nc.scalar.activation(muex3, sst3, AF.Copy)
                        mu, ex3 = muex3[:, 0, :], muex3[:, 1, :]
                        musq = p3.tile([128, LCH], BF16, tag="musq3")
                        nc.gpsimd.tensor_mul(musq, mu, mu)
                        var = p3.tile([128, LCH], BF16, tag="var3")
                        nc.vector.tensor_sub(var, ex3, musq)
                        sd3 = p3.tile([128, LCH], F32, tag="sd3")
                        nc.scalar.activation(sd3, var, AF.Sqrt, bias=EPS)
                        nc.vector.reciprocal(inv2r[:, b, l0:l0 + LCH], sd3)
                        nc.vector.tensor_mul(muinv2r[:, b, l0:l0 + LCH], mu,
                                             inv2r[:, b, l0:l0 + LCH])

              tc.strict_bb_all_engine_barrier()
              # ------- phase 3b: modulate_cm + MLP (Gelu) + gated out -------
              with tc.tile_pool(name="p3b", bufs=2) as p3, \
                  tc.tile_pool(name="p3bq", bufs=2, space="PSUM") as p3q, \
                  tc.tile_pool(name="p3bm", bufs=2, space="PSUM") as p3m, \
                  tc.tile_pool(name="p3bg", bufs=2, space="PSUM") as p3g:
                for ch in range(NCH):
                    for b in range(BPC):
                        l0 = ch * LCH
                        x1 = p3.tile([128, CT, LCH], BF16, tag="x1b")
                        for ct in range(CT):
                            nc.sync.dma_start(
                                out=x1[:, ct, :],
                                in_=x1_d[b, ct * 128:(ct + 1) * 128,
                                         l0:l0 + LCH])
                        inv = inv2r[:, b, l0:l0 + LCH]
                        muinv = muinv2r[:, b, l0:l0 + LCH]
                        z2 = p3.tile([128, CT, LCH], BF16, tag="z2")
                        for ct in range(CT):
                            adp = p3q.tile([128, 2, LCH], F32, tag="adp")
                            bias_mm(adp[:, 0, :], 9 + ct, LCH)
                            MM(adp[:, 0, :], adawT[:, 9 + ct, :],
                               tsl[:, b, l0:l0 + LCH], start=False, stop=True)
                            bias_mm(adp[:, 1, :], 12 + ct, LCH)
                            MM(adp[:, 1, :], adawT[:, 12 + ct, :],
                               tsl[:, b, l0:l0 + LCH], start=False, stop=True)
                            m1 = p3.tile([128, LCH], BF16, tag=f"m13_{ct}")
                            eng = nc.vector if ct != 1 else nc.gpsimd
                            eng.tensor_mul(m1, x1[:, ct, :], inv)
                            z = p3.tile([128, LCH], BF16, tag=f"z3_{ct}")
                            eng2 = nc.gpsimd if ct != 1 else nc.vector
                            eng2.tensor_sub(z, m1, muinv)
                            u = p3.tile([128, LCH], BF16, tag=f"u3_{ct}")
                            if ct != 1:
                                adb = p3.tile([128, 2, LCH], BF16,
                                              tag=f"adb3_{ct}")
                                nc.scalar.activation(adb, adp, AF.Copy)
                                srcS, srcSH = adb[:, 1, :], adb[:, 0, :]
                            else:
                                srcS, srcSH = adp[:, 1, :], adp[:, 0, :]
                            nc.vector.tensor_mul(u, srcS, z)
                            nc.vector.tensor_add(z2[:, ct, :], srcSH, u)
                        # MLP layer 1
                        h = p3.tile([128, CT, LCH], BF16, tag="h")
                        for oc in range(CT):
                            hp = p3m.tile([128, LCH], F32, tag="mlp")
                            for ct in range(CT):
                                MM(hp, w1T[:, ct, oc * 128:(oc + 1) * 128],
                                   z2[:, ct, :], start=(ct == 0),
                                   stop=(ct == CT - 1))
                            nc.scalar.activation(h[:, oc, :], hp, AF.Gelu,
                                                 bias=b1c[:, oc:oc + 1])
                        for oc in range(CT):
                            gcp = p3g.tile([128, LCH], F32, tag="gcs")
                            bias_mm(gcp, 15 + oc, LCH)
                            MM(gcp, adawT[:, 15 + oc, :],
                               tsl[:, b, l0:l0 + LCH], start=False, stop=True)
                            mp = p3m.tile([128, LCH], F32, tag="mlp")
                            for ct in range(CT):
                                MM(mp, w2T[:, ct, oc * 128:(oc + 1) * 128],
                                   h[:, ct, :], start=(ct == 0),
                                   stop=(ct == CT - 1))
                            mb = p3.tile([128, LCH], BF16, tag=f"mb_{oc}")
                            nc.scalar.activation(mb, mp, AF.Identity,
                                                 bias=b2c[:, oc:oc + 1])
                            gc = p3.tile([128, LCH], BF16, tag=f"gc_{oc}")
                            nc.vector.tensor_mul(gc, gcp, mb)
                            ostg = p3.tile([128, LCH], BF16, tag="ostg")
                            eng = (nc.vector, nc.gpsimd, nc.vector)[oc]
                            eng.tensor_add(ostg, x1[:, oc, :], gc)
                            nc.gpsimd.dma_start(
                                out=out_e[b, oc * 128:(oc + 1) * 128,
                                          l0:l0 + LCH],
                                in_=ostg)
    nc.finalize()
    return nc


def kernel(x, t_cond, kernels, D, ada_w, ada_b, w1, b1, w2, b2):
    global _last_results
    consts = _make_consts()
    khre, khim = _make_khat(kernels)
    nc = build_graph()
    shared = {
        "khre": khre,
        "khim": khim,
        "DT": np.ascontiguousarray(
            np.asarray(D, np.float32).reshape(CT, 128).T),
        "ada_wT": np.ascontiguousarray(ada_w.T).astype(ml_dtypes.bfloat16),
        "ada_bT": np.ascontiguousarray(_fold_ada_bias(ada_b)),
        "ada_bR": np.ascontiguousarray(
            _fold_ada_bias(ada_b).T.reshape(1, 18 * 128)
        ).astype(ml_dtypes.bfloat16),
        "w1T": np.ascontiguousarray(w1.T).astype(ml_dtypes.bfloat16),
        "b1T": np.ascontiguousarray(
            np.asarray(b1, np.float32).reshape(CT, 128).T),
        "w2T": np.ascontiguousarray(w2.T).astype(ml_dtypes.bfloat16),
        "b2T": np.ascontiguousarray(
            np.asarray(b2, np.float32).reshape(CT, 128).T),
    }
    shared.update(consts)
    in_maps = []
    for i in range(NCORES):
        m = dict(shared)
        m["x"] = np.ascontiguousarray(x[i * BPC:(i + 1) * BPC], dtype=np.float32)
        m["t_cond"] = np.ascontiguousarray(t_cond[i * BPC:(i + 1) * BPC],
                                           dtype=np.float32)
        in_maps.append(m)
    trace = os.environ.get("KERNEL_TRACE", "0") == "1"
    res = run_bass_kernel_spmd(nc, in_maps, list(range(NCORES)), trace=trace)
    _last_results = res
    outs = [r["out"] if isinstance(r, dict) else r for r in res.results]
    return np.concatenate([np.asarray(o, dtype=np.float32).reshape(BPC, C, L)
                           for o in outs], axis=0)


if __name__ == "__main__":
    build_graph()
    print("graph built ok")



# revision 41
# speedup vs baseline: 1.2926x; 1.0041x over previous
"""AdaConvBlock Trainium2 kernel: 8-core data-parallel (2 batch elems/core).

Per core (b=2, C=384, L=4096):
  LN1 -> adaLN modulate -> SLConv (1024-tap depthwise conv via four-step
  matmul FFT, N=4608=128x36, batch pair packed as complex) + D-skip ->
  gated residual -> LN2 -> modulate -> pointwise MLP (gelu) -> gated residual.

FFT: Cooley-Tukey twiddles folded into 36 per-n2 stationary matrices ->
pure matmul FFT, no twiddle pointwise work. PSUM is managed in explicit
2KB-bank slots (one matmul output per slot, has_written gives per-slot
overwrite/accumulate semantics).
"""
import os
import sys

sys.path.insert(0, "/opt/trn_rl_repo")

import numpy as np
import ml_dtypes

import concourse.bass as bass
import concourse.bacc as bacc
import concourse.tile as tile
from concourse import mybir
from concourse.bass_utils import run_bass_kernel_spmd

F32 = mybir.dt.float32
BF16 = mybir.dt.bfloat16
AX = mybir.AluOpType
AF = mybir.ActivationFunctionType

B, C, L = 16, 384, 4096
NCORES = 8
BPC = B // NCORES
CT = 3
NCH = 8
LCH = 512
N, N1, N2 = 4608, 128, 36
KLEN = 1024
SS = 512
NG = 8
GS = 48
NSUB = 16
EPS = 1e-5
DECAY = 2.0
NS, KS = 6, 32

_last_results = None


def _make_consts():
    k1 = np.arange(N1)
    n1 = np.arange(N1)
    W128 = np.exp(-2j * np.pi * np.outer(k1, n1) / N1)
    W36 = np.exp(-2j * np.pi * np.outer(np.arange(N2), np.arange(N2)) / N2)
    fA = np.zeros((N2, N1, N1), complex)
    iA = np.zeros((N2, N1, N1), complex)
    for n2 in range(N2):
        M = np.exp(-2j * np.pi * n2 * k1 / N)[:, None] * W128      # [k1,n1]
        fA[n2] = M.T                                               # lhsT [n1,k1]
        IA = (np.exp(2j * np.pi * n2 * k1 / N)[None, :] * np.conj(W128).T) / N
        iA[n2] = IA.T                                              # lhsT [k1,n1]

    def blockdiag3(Mx):
        out = np.zeros((108, 108), complex)
        for s in range(3):
            out[s * 36:(s + 1) * 36, s * 36:(s + 1) * 36] = Mx
        return out

    fB = blockdiag3(W36.T)
    iB = blockdiag3(np.conj(W36).T)

    def bf(x):
        return np.ascontiguousarray(x).astype(ml_dtypes.bfloat16)

    return {
        "fA_re": bf(fA.real), "fA_im": bf(fA.imag), "fA_imn": bf(-fA.imag),
        "iA_re": bf(iA.real), "iA_im": bf(iA.imag), "iA_imn": bf(-iA.imag),
        "fB_re": bf(fB.real), "fB_im": bf(fB.imag), "fB_imn": bf(-fB.imag),
        "iB_re": bf(iB.real), "iB_im": bf(iB.imag), "iB_imn": bf(-iB.imag),
        "ident": bf(np.eye(128)),
        "ones_bf": bf(np.ones((128, 128))),
        "ones_c": bf(np.full((128, 128), 1.0 / C)),
        "ones_row": bf(np.ones((1, 1024))),
    }


def _make_khat(kernels):
    """Host-side kernel FFT: build the normalized multi-scale kernel, FFT to
    length N=4608, and lay out per group in the spectral-domain tile layout
    produced by the on-device fwd FFT: KH[g, c_loc*36+k2, s*128+k1] =
    FFT(k_c)[k1 + 128*k2] with c = 48g + 3s + c_loc."""
    ker = np.asarray(kernels, np.float64)  # (NS, 1, C, KS)
    klist = [np.repeat(ker[i, 0], 2 ** max(0, i - 1), axis=-1)
             * (DECAY ** (NS - i - 1)) for i in range(NS)]
    k = np.concatenate(klist, axis=-1)  # (C, KLEN)
    k = k / np.linalg.norm(k, axis=-1, keepdims=True)
    Ksp = np.fft.fft(k, N, axis=-1)  # (C, N)
    Kp = Ksp.reshape(C, N2, N1)                      # [c, k2, k1]
    Kp = Kp.reshape(NG, GS // 3, 3, N2, N1)          # [g, s, c_loc, k2, k1]
    Kp = Kp.transpose(0, 2, 3, 1, 4).reshape(NG, 108, NSUB * 128)

    def bf(x):
        return np.ascontiguousarray(x).astype(ml_dtypes.bfloat16)

    return bf(Kp.real), bf(Kp.imag)


# bank-slot offset for per-n2 FFT matmul outputs: 10 slots of 48 per 2KB bank
def _n2off(n2):
    return (n2 // 10) * 512 + (n2 % 10) * 48


def _fold_ada_bias(ada_b):
    """[128, 18] bias columns; scale chunks (3..5 tm, 12..14 cm) get +1 so
    modulate is y = z*scale' + shift with scale' = 1 + scale."""
    ab = np.asarray(ada_b, np.float32).reshape(18, 128).T.copy()
    ab[:, 3:6] += 1.0
    ab[:, 12:15] += 1.0
    return ab


def build_graph():
    nc = bacc.Bacc(None)

    x_e = nc.declare_dram_parameter("x", [BPC, C, L], F32, isOutput=False)
    tc_e = nc.declare_dram_parameter("t_cond", [BPC, C // 3, L], F32, isOutput=False)
    khre_e = nc.declare_dram_parameter("khre", [NG, 108, NSUB * 128], BF16,
                                       isOutput=False)
    khim_e = nc.declare_dram_parameter("khim", [NG, 108, NSUB * 128], BF16,
                                       isOutput=False)
    d_e = nc.declare_dram_parameter("DT", [128, CT], F32, isOutput=False)
    adawT_e = nc.declare_dram_parameter("ada_wT", [C // 3, 6 * C], BF16, isOutput=False)
    adab_e = nc.declare_dram_parameter("ada_bT", [128, 18], F32, isOutput=False)
    adabR_e = nc.declare_dram_parameter("ada_bR", [1, 18 * 128], BF16,
                                        isOutput=False)
    FP8 = mybir.dt.float8e4
    w1T_e = nc.declare_dram_parameter("w1T", [C, C], FP8, isOutput=False)
    b1_e = nc.declare_dram_parameter("b1T", [128, CT], F32, isOutput=False)
    w2T_e = nc.declare_dram_parameter("w2T", [C, C], FP8, isOutput=False)
    b2_e = nc.declare_dram_parameter("b2T", [128, CT], F32, isOutput=False)
    cshapes = {
        "fA_re": [N2, N1, N1], "fA_im": [N2, N1, N1], "fA_imn": [N2, N1, N1],
        "iA_re": [N2, N1, N1], "iA_im": [N2, N1, N1], "iA_imn": [N2, N1, N1],
        "fB_re": [108, 108], "fB_im": [108, 108], "fB_imn": [108, 108],
        "iB_re": [108, 108], "iB_im": [108, 108], "iB_imn": [108, 108],
        "ident": [128, 128], "ones_bf": [128, 128],
        "ones_c": [128, 128], "ones_row": [1, 1024],
    }
    cst = {nm: nc.declare_dram_parameter(nm, shp, BF16, isOutput=False)
           for nm, shp in cshapes.items()}
    out_e = nc.declare_dram_parameter("out", [BPC, C, L], BF16, isOutput=True)

    ymod_d = nc.dram_tensor("ymod", [BPC, C, N], BF16)
    yconv_d = nc.dram_tensor("yconv", [BPC, C, N], BF16)
    x1_d = nc.dram_tensor("x1s", [BPC, C, L], BF16)

    MM = nc.tensor.matmul

    with tile.TileContext(nc) as tc, \
         nc.allow_low_precision(reason="bf16 datapath, fp32 psum accumulation"), \
         tc.tile_pool(name="sing", bufs=1) as sing:
        if True:
            zc = sing.tile([128, 1], F32)
            nc.vector.memset(zc, 0.0)
            nc.const_aps.aps[(F32, 0.0)] = zc[:, :]
            ec = sing.tile([128, 1], F32)
            nc.vector.memset(ec, EPS)
            nc.const_aps.aps[(F32, EPS)] = ec[:, :]
            sb = {}
            qengs = (nc.sync, nc.gpsimd, nc.scalar)
            for nm in ("fA_re", "fA_im", "fA_imn", "iA_re", "iA_im", "iA_imn"):
                sb[nm] = sing.tile([N1, N2, N1], BF16, tag=nm, name=nm)
            for nm in ("fB_re", "fB_im", "fB_imn", "iB_re", "iB_im", "iB_imn"):
                sb[nm] = sing.tile([108, 108], BF16, tag=nm, name=nm)
            ident = sing.tile([128, 128], BF16)
            ident32 = sing.tile([128, 128], F32)

            def load_fft_consts():
                # issued mid-ph1: 43us of const DMA overlaps ph1 compute
                for qi, nm in enumerate(("fA_re", "fA_im", "fA_imn", "iA_re",
                                         "iA_im", "iA_imn")):
                    qengs[qi % 3].dma_start(
                        out=sb[nm], in_=cst[nm].rearrange("a b c -> b a c"))
                for qi, nm in enumerate(("fB_re", "fB_im", "fB_imn", "iB_re",
                                         "iB_im", "iB_imn")):
                    qengs[qi % 3].dma_start(out=sb[nm], in_=cst[nm][:, :])
                nc.scalar.dma_start(out=ident, in_=cst["ident"][:, :])
                nc.scalar.activation(ident32, ident, AF.Copy)
            ones_bf = sing.tile([128, 128], BF16)
            nc.sync.dma_start(out=ones_bf, in_=cst["ones_bf"][:, :])
            ones_c = sing.tile([128, 128], BF16)
            nc.gpsimd.dma_start(out=ones_c, in_=cst["ones_c"][:, :])
            ones_row = sing.tile([1, 1024], BF16)
            nc.gpsimd.dma_start(out=ones_row, in_=cst["ones_row"][:, :])
            adabR = sing.tile([1, 18, 128], BF16)
            nc.gpsimd.dma_start(out=adabR,
                                in_=adabR_e.rearrange("a (c o) -> a c o", c=18))

            def bias_mm(psl, ch, lch):
                # K=1 matmul: adds ada bias column for chunk ch across lch
                MM(psl, adabR[0:1, ch, :], ones_row[0:1, 0:lch],
                   start=True, stop=False)
            adawT = sing.tile([128, 18, 128], BF16)
            nc.sync.dma_start(out=adawT,
                              in_=adawT_e.rearrange("k (c o) -> k c o", c=18))
            adab = sing.tile([128, 18], F32)
            nc.sync.dma_start(out=adab, in_=adab_e[:, :])
            w1T = sing.tile([128, CT, C], FP8)
            nc.sync.dma_start(out=w1T,
                              in_=w1T_e.rearrange("(a k) o -> k a o", k=128))
            w2T = sing.tile([128, CT, C], FP8)
            nc.sync.dma_start(out=w2T,
                              in_=w2T_e.rearrange("(a k) o -> k a o", k=128))
            b1c = sing.tile([128, CT], F32)
            nc.sync.dma_start(out=b1c, in_=b1_e[:, :])
            b2c = sing.tile([128, CT], F32)
            nc.sync.dma_start(out=b2c, in_=b2_e[:, :])
            dcol = sing.tile([128, CT], F32)
            nc.sync.dma_start(out=dcol, in_=d_e[:, :])

            # silu(t_cond) computed once, resident for ph1 (ada) and ph3
            tsl = sing.tile([128, BPC, L], BF16, tag="tsl")
            with tc.tile_pool(name="p0", bufs=2) as p0:
                for b in range(BPC):
                    tcf = p0.tile([128, L], F32, tag="tcf")
                    nc.sync.dma_start(out=tcf, in_=tc_e[b, :, :])
                    nc.scalar.activation(tsl[:, b, :], tcf, AF.Silu)

            import os as _os
            _STAGES = int(_os.environ.get("KSTAGES", "4"))
            # ---------------- phase 1: LN1 + modulate -> ymod -------
            # NOTE: ada_bT scale chunks (3..5, 12..14) carry a host-folded +1
            # so modulate is y = z*(scale') + shift with scale' = 1+scale.
            if _STAGES >= 2:
             with tc.tile_pool(name="p1", bufs=3) as p1, \
                 tc.tile_pool(name="p1p", bufs=1, space="PSUM") as p1p, \
                 tc.tile_pool(name="p1q", bufs=3, space="PSUM") as p1q:
                zpadN = p1.tile([128, N - L], BF16, tag="zpadN")
                nc.vector.memset(zpadN, 0.0)
                for b in range(BPC):
                    for ct in range(CT):
                        nc.gpsimd.dma_start(
                            out=ymod_d[b, ct * 128:(ct + 1) * 128, L:N],
                            in_=zpadN)
                for b in range(BPC):
                    for ch in range(NCH):
                        if b == 0 and ch == 2:
                            load_fft_consts()
                        l0 = ch * LCH
                        xf = p1.tile([128, CT, LCH], F32, tag="xf")
                        for ct in range(CT):
                            dq = nc.sync if ct != 1 else nc.gpsimd
                            dq.dma_start(
                                out=xf[:, ct, :],
                                in_=x_e[b, ct * 128:(ct + 1) * 128, l0:l0 + LCH])
                        xb = p1.tile([128, CT, LCH], BF16, tag="xb")
                        x2 = p1.tile([128, CT, LCH], BF16, tag="x2")
                        for ct in range(CT):
                            if ct == 2:
                                nc.vector.tensor_copy(xb[:, ct, :], xf[:, ct, :])
                            else:
                                nc.scalar.activation(xb[:, ct, :], xf[:, ct, :],
                                                     AF.Copy)
                            eng = nc.vector if ct != 0 else nc.gpsimd
                            eng.tensor_mul(x2[:, ct, :], xb[:, ct, :],
                                           xb[:, ct, :])
                        sst = p1p.tile([128, 2, LCH], F32, tag="sst")
                        for ct in range(CT):
                            MM(sst[:, 0, :], ones_c, xb[:, ct, :],
                               start=(ct == 0), stop=(ct == CT - 1))
                        for ct in range(CT):
                            MM(sst[:, 1, :], ones_c, x2[:, ct, :],
                               start=(ct == 0), stop=(ct == CT - 1))
                        muex = p1.tile([128, 2, LCH], BF16, tag="muex")
                        nc.scalar.activation(muex, sst, AF.Copy)
                        mu, ex = muex[:, 0, :], muex[:, 1, :]
                        musq = p1.tile([128, LCH], BF16, tag="musq")
                        nc.gpsimd.tensor_mul(musq, mu, mu)
                        var = p1.tile([128, LCH], BF16, tag="var")
                        nc.vector.tensor_sub(var, ex, musq)
                        sd_ = p1.tile([128, LCH], F32, tag="sd_")
                        nc.scalar.activation(sd_, var, AF.Sqrt, bias=EPS)
                        inv = p1.tile([128, LCH], BF16, tag="inv")
                        nc.vector.reciprocal(inv, sd_)
                        muinv = p1.tile([128, LCH], BF16, tag="muinv")
                        nc.vector.tensor_mul(muinv, mu, inv)
                        ym = p1.tile([128, CT, LCH], BF16, tag="ym")
                        for ct in range(CT):
                            adp = p1q.tile([128, 2, LCH], F32, tag="adp")
                            bias_mm(adp[:, 0, :], ct, LCH)
                            MM(adp[:, 0, :], adawT[:, ct, :],
                               tsl[:, b, l0:l0 + LCH], start=False, stop=True)
                            bias_mm(adp[:, 1, :], 3 + ct, LCH)
                            MM(adp[:, 1, :], adawT[:, 3 + ct, :],
                               tsl[:, b, l0:l0 + LCH], start=False, stop=True)
                            m1 = p1.tile([128, LCH], BF16, tag=f"m1_{ct}")
                            eng = nc.vector if ct != 1 else nc.gpsimd
                            eng.tensor_mul(m1, xb[:, ct, :], inv)
                            z = p1.tile([128, LCH], BF16, tag=f"z_{ct}")
                            eng2 = nc.gpsimd if ct != 1 else nc.vector
                            eng2.tensor_sub(z, m1, muinv)
                            u = p1.tile([128, LCH], BF16, tag=f"u_{ct}")
                            if ct != 1:
                                # drain biased PSUM via Act copy; TT at 2x
                                adb = p1.tile([128, 2, LCH], BF16,
                                              tag=f"adb_{ct}")
                                nc.scalar.activation(adb, adp, AF.Copy)
                                srcS, srcSH = adb[:, 1, :], adb[:, 0, :]
                            else:
                                srcS, srcSH = adp[:, 1, :], adp[:, 0, :]
                            nc.vector.tensor_mul(u, srcS, z)
                            nc.vector.tensor_add(ym[:, ct, :], srcSH, u)
                        for ct in range(CT):
                            nc.gpsimd.dma_start(
                                out=ymod_d[b, ct * 128:(ct + 1) * 128,
                                           l0:l0 + LCH],
                                in_=ym[:, ct, :])

            tc.strict_bb_all_engine_barrier()
            # ---------------- phase 2: kernel FFT + conv FFT --------
            if _STAGES >= 3:
             with tc.tile_pool(name="p2", bufs=1) as p2, \
                 tc.tile_pool(name="p2in", bufs=2) as p2in, \
                 tc.tile_pool(name="p2p", bufs=1, space="PSUM") as p2p:

                H2 = [(0, 20), (20, 16)]    # n2 halves (start, count)
                HS = [(0, 8), (8, 8)]        # subgroup halves

                def pair(h):
                    t = "a" if h == 0 else "b"
                    pre = p2p.tile([128, 1024], F32, tag=f"p{t}_re")
                    pim = p2p.tile([128, 1024], F32, tag=f"p{t}_im")
                    return pre, pim

                def unscr_half(dst, psrc, h, eng):
                    # psrc [128,1024]: 2 banks of 10 slots x 48 -> dst ch-major
                    d3 = dst.rearrange("p (c n) -> p c n", n=N2)
                    st, cnt = H2[h]
                    s5 = psrc.rearrange("p (bk r) -> p bk r", bk=2)[
                        :, :, 0:480].rearrange("p bk (sl c) -> p bk sl c",
                                               sl=10)
                    act = eng is nc.scalar
                    if cnt == 20:
                        o = d3[:, :, st:st + 20].rearrange(
                            "p c (bk sl) -> p bk sl c", bk=2)
                        if act:
                            eng.activation(o, s5[:, :, :, 0:48], AF.Copy)
                        else:
                            eng.tensor_copy(o, s5[:, :, :, 0:48])
                    else:
                        o1 = d3[:, :, st:st + 10].rearrange("p c n -> p n c")
                        o2 = d3[:, :, st + 10:st + 16].rearrange(
                            "p c n -> p n c")
                        if act:
                            eng.activation(o1, s5[:, 0, :, 0:48], AF.Copy)
                            eng.activation(o2, s5[:, 1, 0:6, 0:48], AF.Copy)
                        else:
                            eng.tensor_copy(o1, s5[:, 0, :, 0:48])
                            eng.tensor_copy(o2, s5[:, 1, 0:6, 0:48])

                def f1_half(pre, pim, h, zr, zi, real):
                    st, cnt = H2[h]
                    for jx in range(cnt):
                        n2 = st + jx
                        off = (jx // 10) * 512 + (jx % 10) * 48
                        if real:
                            MM(pre[:, off:off + GS], sb["fA_re"][0:29, n2, :],
                               zr[:, :, n2], start=True, stop=True)
                            MM(pim[:, off:off + GS], sb["fA_im"][0:29, n2, :],
                               zr[:, :, n2], start=True, stop=True)
                        else:
                            MM(pre[:, off:off + GS], sb["fA_re"][:, n2, :],
                               zr[:, :, n2], start=True, stop=False)
                            MM(pim[:, off:off + GS], sb["fA_im"][:, n2, :],
                               zr[:, :, n2], start=True, stop=False)
                            MM(pre[:, off:off + GS], sb["fA_imn"][:, n2, :],
                               zi[:, :, n2], start=False, stop=True)
                            MM(pim[:, off:off + GS], sb["fA_re"][:, n2, :],
                               zi[:, :, n2], start=False, stop=True)

                def i4_half(pre, pim, h, vr3, vi3):
                    st, cnt = H2[h]
                    for jx in range(cnt):
                        n2 = st + jx
                        off = (jx // 10) * 512 + (jx % 10) * 48
                        MM(pre[:, off:off + GS], sb["iA_re"][:, n2, :],
                           vr3[:, :, n2], start=True, stop=False)
                        MM(pim[:, off:off + GS], sb["iA_im"][:, n2, :],
                           vr3[:, :, n2], start=True, stop=False)
                        MM(pre[:, off:off + GS], sb["iA_imn"][:, n2, :],
                           vi3[:, :, n2], start=False, stop=True)
                        MM(pim[:, off:off + GS], sb["iA_re"][:, n2, :],
                           vi3[:, :, n2], start=False, stop=True)

                def t_half(pre, pim, h, inre, inim):
                    st, _ = HS[h]
                    for s in range(st, st + 8):
                        off = ((s - st) // 4) * 512 + ((s - st) % 4) * 128
                        isl = slice(s * 108, (s + 1) * 108)
                        MM(pre[:108, off:off + 128], inre[:, isl], ident32,
                           is_transpose=True, start=True, stop=True)
                        MM(pim[:108, off:off + 128], inim[:, isl], ident32,
                           is_transpose=True, start=True, stop=True)

                def tb_half(pre, pim, h, inre, inim):
                    st, _ = HS[h]
                    for s in range(st, st + 8):
                        off = ((s - st) // 4) * 512 + ((s - st) % 4) * 108
                        isl = slice(s * 128, (s + 1) * 128)
                        MM(pre[:, off:off + 108], inre[:108, isl],
                           ident32[:108, :108], is_transpose=True, start=True,
                           stop=True)
                        MM(pim[:, off:off + 108], inim[:108, isl],
                           ident32[:108, :108], is_transpose=True, start=True,
                           stop=True)

                def d36_half(pre, pim, h, Bre, Bim, Bimn, inre, inim):
                    st, _ = HS[h]
                    for s in range(st, st + 8):
                        off = (s - st) * 128
                        sl = slice(s * 128, (s + 1) * 128)
                        MM(pre[:108, off:off + 128], Bre, inre[:, sl],
                           start=True, stop=False)
                        MM(pim[:108, off:off + 128], Bim, inre[:, sl],
                           start=True, stop=False)
                        MM(pre[:108, off:off + 128], Bimn, inim[:, sl],
                           start=False, stop=True)
                        MM(pim[:108, off:off + 128], Bre, inim[:, sl],
                           start=False, stop=True)

                def hcopy(dst, psrc, h, eng):
                    o = dst[:, h * 1024:(h + 1) * 1024]
                    if eng is nc.scalar:
                        eng.activation(o, psrc[:108, :], AF.Copy)
                    else:
                        eng.tensor_copy(o, psrc[:108, :])

                def vcopy_half(vflat, psrc, h, eng):
                    for bk in range(2):
                        o = vflat[:, (h * 8 + bk * 4) * 108:
                                  (h * 8 + bk * 4) * 108 + 432]
                        s_ = psrc[:, bk * 512:bk * 512 + 432]
                        if eng is nc.scalar:
                            eng.activation(o, s_, AF.Copy)
                        else:
                            eng.tensor_copy(o, s_)

                for g in range(NG):
                    c0 = g * GS
                    # ======== kernel FFT: precomputed host-side ========
                    kh_re = p2in.tile([108, NSUB * 128], BF16, tag="kh_re")
                    kh_im = p2in.tile([108, NSUB * 128], BF16, tag="kh_im")
                    nc.gpsimd.dma_start(out=kh_re, in_=khre_e[g])
                    nc.gpsimd.dma_start(out=kh_im, in_=khim_e[g])

                    # ======== data FFT (batch pair packed complex) ========
                    z_re = p2in.tile([128, GS, N2], BF16, tag="z_re")
                    z_im = p2in.tile([128, GS, N2], BF16, tag="z_im")
                    nc.sync.dma_start(
                        out=z_re, in_=ymod_d[0, c0:c0 + GS, :].rearrange(
                            "c (a b) -> a c b", b=N2))
                    nc.sync.dma_start(
                        out=z_im, in_=ymod_d[1, c0:c0 + GS, :].rearrange(
                            "c (a b) -> a c b", b=N2))
                    S_re = p2.tile([128, GS * N2], F32, tag="S_re")
                    S_im = p2.tile([128, GS * N2], F32, tag="S_im")
                    for h in (0, 1):
                        pre, pim = pair(h)
                        f1_half(pre, pim, h, z_re, z_im, False)
                        unscr_half(S_re, pre, h, nc.scalar)
                        unscr_half(S_im, pim, h, nc.vector)
                    ST_re = p2.tile([108, NSUB * 128], BF16, tag="ST_re")
                    ST_im = p2.tile([108, NSUB * 128], BF16, tag="ST_im")
                    for h in (0, 1):
                        pre, pim = pair(h)
                        t_half(pre, pim, h, S_re, S_im)
                        hcopy(ST_re, pre, h, nc.scalar)
                        hcopy(ST_im, pim, h, nc.vector)
                    X_re = p2.tile([108, NSUB * 128], BF16, tag="X_re")
                    X_im = p2.tile([108, NSUB * 128], BF16, tag="X_im")
                    for h in (0, 1):
                        pre, pim = pair(h)
                        d36_half(pre, pim, h, sb["fB_re"], sb["fB_im"],
                                 sb["fB_imn"], ST_re, ST_im)
                        hcopy(X_re, pre, h, nc.scalar)
                        hcopy(X_im, pim, h, nc.vector)
                    # spectral multiply (per half to keep pipeline fine-grained)
                    Y_re = p2.tile([108, NSUB * 128], BF16, tag="Y_re")
                    Y_im = p2.tile([108, NSUB * 128], BF16, tag="Y_im")
                    q1 = p2.tile([108, NSUB * 128], BF16, tag="q1")
                    q2 = p2.tile([108, NSUB * 128], BF16, tag="q2")
                    for h in (0, 1):
                        sl = slice(h * 1024, (h + 1) * 1024)
                        nc.vector.tensor_mul(q1[:, sl], X_re[:, sl],
                                             kh_re[:, sl])
                        nc.gpsimd.tensor_mul(q2[:, sl], X_im[:, sl],
                                             kh_im[:, sl])
                        nc.gpsimd.tensor_sub(Y_re[:, sl], q1[:, sl],
                                             q2[:, sl])
                        nc.vector.tensor_mul(q1[:, sl], X_re[:, sl],
                                             kh_im[:, sl])
                        nc.gpsimd.tensor_mul(q2[:, sl], X_im[:, sl],
                                             kh_re[:, sl])
                        nc.vector.tensor_add(Y_im[:, sl], q1[:, sl],
                                             q2[:, sl])
                    U_re = p2.tile([108, NSUB * 128], F32, tag="U_re")
                    U_im = p2.tile([108, NSUB * 128], F32, tag="U_im")
                    for h in (0, 1):
                        pre, pim = pair(h)
                        d36_half(pre, pim, h, sb["iB_re"], sb["iB_im"],
                                 sb["iB_imn"], Y_re, Y_im)
                        hcopy(U_re, pre, h, nc.scalar)
                        hcopy(U_im, pim, h, nc.vector)
                    V_re = p2.tile([128, GS, N2], BF16, tag="V_re")
                    V_im = p2.tile([128, GS, N2], BF16, tag="V_im")
                    vr = V_re.rearrange("p a b -> p (a b)")
                    vi = V_im.rearrange("p a b -> p (a b)")
                    for h in (0, 1):
                        pre, pim = pair(h)
                        tb_half(pre, pim, h, U_re, U_im)
                        vcopy_half(vr, pre, h, nc.scalar)
                        vcopy_half(vi, pim, h, nc.vector)
                    yo_re = p2.tile([128, GS, N2], BF16, tag="yo_re")
                    yo_im = p2.tile([128, GS, N2], BF16, tag="yo_im")
                    yof_re = yo_re.rearrange("p a b -> p (a b)")
                    yof_im = yo_im.rearrange("p a b -> p (a b)")
                    for h in (0, 1):
                        pre, pim = pair(h)
                        i4_half(pre, pim, h, V_re, V_im)
                        unscr_half(yof_re, pre, h, nc.scalar)
                        unscr_half(yof_im, pim, h, nc.vector)
                    nc.gpsimd.dma_start(
                        out=yconv_d[0, c0:c0 + GS, :].rearrange(
                            "c (a b) -> a c b", b=N2), in_=yo_re)
                    nc.gpsimd.dma_start(
                        out=yconv_d[1, c0:c0 + GS, :].rearrange(
                            "c (a b) -> a c b", b=N2), in_=yo_im)

            tc.strict_bb_all_engine_barrier()
            # ------- phase 3a: residual + gate_tm + LN2 stats (Rsqrt) -------
            # inv2/muinv2 kept SBUF-resident for ph3b; x1 staged via DRAM.
            if _STAGES >= 4:
             with tc.tile_pool(name="p3r", bufs=1) as p3r:
              inv2r = p3r.tile([128, BPC, L], BF16, tag="inv2r")
              muinv2r = p3r.tile([128, BPC, L], BF16, tag="muinv2r")
              with tc.tile_pool(name="p3a", bufs=2) as p3, \
                  tc.tile_pool(name="p3ap", bufs=1, space="PSUM") as p3p, \
                  tc.tile_pool(name="p3aq", bufs=3, space="PSUM") as p3pm:
                for ch in range(NCH):
                    for b in range(BPC):
                        l0 = ch * LCH
                        yc = p3.tile([128, CT, LCH], BF16, tag="yc3")
                        ym3 = p3.tile([128, CT, LCH], BF16, tag="ym3")
                        xf3 = p3.tile([128, CT, LCH], F32, tag="xf3")
                        for ct in range(CT):
                            nc.sync.dma_start(
                                out=xf3[:, ct, :],
                                in_=x_e[b, ct * 128:(ct + 1) * 128, l0:l0 + LCH])
                            nc.sync.dma_start(
                                out=yc[:, ct, :],
                                in_=yconv_d[b, ct * 128:(ct + 1) * 128,
                                            SS + l0:SS + l0 + LCH])
                            nc.gpsimd.dma_start(
                                out=ym3[:, ct, :],
                                in_=ymod_d[b, ct * 128:(ct + 1) * 128,
                                           l0:l0 + LCH])
                        x1 = p3.tile([128, CT, LCH], BF16, tag="x1")
                        x2t = p3.tile([128, CT, LCH], BF16, tag="x2t")
                        for ct in range(CT):
                            adp3 = p3pm.tile([128, LCH], F32, tag="adp3")
                            bias_mm(adp3, 6 + ct, LCH)
                            MM(adp3, adawT[:, 6 + ct, :],
                               tsl[:, b, l0:l0 + LCH], start=False, stop=True)
                            # s1 = D*ym + yconv: tensor_scalar (4x) + TT (2x)
                            dm = p3.tile([128, LCH], BF16, tag=f"dm_{ct}")
                            nc.vector.tensor_scalar(
                                dm, ym3[:, ct, :], dcol[:, ct:ct + 1],
                                None, AX.mult)
                            s1 = p3.tile([128, LCH], BF16, tag=f"s1_{ct}")
                            eng0 = nc.gpsimd if ct == 1 else nc.vector
                            eng0.tensor_add(s1, dm, yc[:, ct, :])
                            # gx = gate_tm' * s1 (bias folded into PSUM)
                            gx = p3.tile([128, LCH], BF16, tag=f"gx_{ct}")
                            if ct != 1:
                                gtb = p3.tile([128, LCH], BF16, tag=f"gtb_{ct}")
                                nc.scalar.activation(gtb, adp3, AF.Copy)
                                nc.vector.tensor_mul(gx, gtb, s1)
                            else:
                                nc.vector.tensor_mul(gx, adp3, s1)
                            eng = nc.vector if ct == 1 else nc.gpsimd
                            eng.tensor_add(x1[:, ct, :], xf3[:, ct, :], gx)
                            nc.scalar.activation(x2t[:, ct, :], x1[:, ct, :],
                                                 AF.Square)
                            nc.gpsimd.dma_start(
                                out=x1_d[b, ct * 128:(ct + 1) * 128,
                                         l0:l0 + LCH],
                                in_=x1[:, ct, :])
                        sst3 = p3p.tile([128, 2, LCH], F32, tag="sst3")
                        for ct in range(CT):
                            MM(sst3[:, 0, :], ones_c, x1[:, ct, :],
                               start=(ct == 0), stop=(ct == CT - 1))
                        for ct in range(CT):
                            MM(sst3[:, 1, :], ones_c, x2t[:, ct, :],
                               start=(ct == 0), stop=(ct == CT - 1))
                        muex3 = p3.tile([128, 2, LCH], BF16, tag="muex3")
                        nc.scalar.activation(muex3, sst3, AF.Copy)
                        mu, ex3 = muex3[:, 0, :], muex3[:, 1, :]
                        musq = p3.tile([128, LCH], BF16, tag="musq3")
                        nc.gpsimd.tensor_mul(musq, mu, mu)
                        var = p3.tile([128, LCH], BF16, tag="var3")
                        nc.vector.tensor_sub(var, ex3, musq)
                        sd3 = p3.tile([128, LCH], F32, tag="sd3")
                        nc.scalar.activation(sd3, var, AF.Sqrt, bias=EPS)
                        nc.vector.reciprocal(inv2r[:, b, l0:l0 + LCH], sd3)
                        nc.vector.tensor_mul(muinv2r[:, b, l0:l0 + LCH], mu,
                                             inv2r[:, b, l0:l0 + LCH])

              tc.strict_bb_all_engine_barrier()
              # ------- phase 3b: modulate_cm + MLP (Gelu) + gated out -------
              with tc.tile_pool(name="p3b", bufs=2) as p3, \
                  tc.tile_pool(name="p3bq", bufs=2, space="PSUM") as p3q, \
                  tc.tile_pool(name="p3bm", bufs=2, space="PSUM") as p3m, \
                  tc.tile_pool(name="p3bg", bufs=2, space="PSUM") as p3g:
                for ch in range(NCH):
                    for b in range(BPC):
                        l0 = ch * LCH
                        x1 = p3.tile([128, CT, LCH], BF16, tag="x1b")
                        for ct in range(CT):
                            nc.sync.dma_start(
                                out=x1[:, ct, :],
                                in_=x1_d[b, ct * 128:(ct + 1) * 128,
                                         l0:l0 + LCH])
                        inv = inv2r[:, b, l0:l0 + LCH]
                        muinv = muinv2r[:, b, l0:l0 + LCH]
                        z2 = p3.tile([128, CT, LCH], FP8, tag="z2")
                        for ct in range(CT):
                            adp = p3q.tile([128, 2, LCH], F32, tag="adp")
                            bias_mm(adp[:, 0, :], 9 + ct, LCH)
                            MM(adp[:, 0, :], adawT[:, 9 + ct, :],
                               tsl[:, b, l0:l0 + LCH], start=False, stop=True)
                            bias_mm(adp[:, 1, :], 12 + ct, LCH)
                            MM(adp[:, 1, :], adawT[:, 12 + ct, :],
                               tsl[:, b, l0:l0 + LCH], start=False, stop=True)
                            m1 = p3.tile([128, LCH], BF16, tag=f"m13_{ct}")
                            eng = nc.vector if ct != 1 else nc.gpsimd
                            eng.tensor_mul(m1, x1[:, ct, :], inv)
                            z = p3.tile([128, LCH], BF16, tag=f"z3_{ct}")
                            eng2 = nc.gpsimd if ct != 1 else nc.vector
                            eng2.tensor_sub(z, m1, muinv)
                            u = p3.tile([128, LCH], BF16, tag=f"u3_{ct}")
                            if ct != 1:
                                adb = p3.tile([128, 2, LCH], BF16,
                                              tag=f"adb3_{ct}")
                                nc.scalar.activation(adb, adp, AF.Copy)
                                srcS, srcSH = adb[:, 1, :], adb[:, 0, :]
                            else:
                                srcS, srcSH = adp[:, 1, :], adp[:, 0, :]
                            nc.vector.tensor_mul(u, srcS, z)
                            nc.vector.tensor_add(z2[:, ct, :], srcSH, u)
                        # MLP layer 1
                        h = p3.tile([128, CT, LCH], FP8, tag="h")
                        for oc in range(CT):
                            hp = p3m.tile([128, LCH], F32, tag="mlp")
                            MM(hp, w1T[:, 0:2, oc * 128:(oc + 1) * 128],
                               z2[:, 0:2, :], start=True, stop=False,
                               perf_mode=mybir.MatmulPerfMode.DoubleRow)
                            MM(hp, w1T[:, 2, oc * 128:(oc + 1) * 128],
                               z2[:, 2, :], start=False, stop=True)
                            nc.scalar.activation(h[:, oc, :], hp, AF.Gelu,
                                                 bias=b1c[:, oc:oc + 1])
                        for oc in range(CT):
                            gcp = p3g.tile([128, LCH], F32, tag="gcs")
                            bias_mm(gcp, 15 + oc, LCH)
                            MM(gcp, adawT[:, 15 + oc, :],
                               tsl[:, b, l0:l0 + LCH], start=False, stop=True)
                            mp = p3m.tile([128, LCH], F32, tag="mlp")
                            MM(mp, w2T[:, 0:2, oc * 128:(oc + 1) * 128],
                               h[:, 0:2, :], start=True, stop=False,
                               perf_mode=mybir.MatmulPerfMode.DoubleRow)
                            MM(mp, w2T[:, 2, oc * 128:(oc + 1) * 128],
                               h[:, 2, :], start=False, stop=True)
                            mb = p3.tile([128, LCH], BF16, tag=f"mb_{oc}")
                            nc.scalar.activation(mb, mp, AF.Identity,
                                                 bias=b2c[:, oc:oc + 1])
                            gc = p3.tile([128, LCH], BF16, tag=f"gc_{oc}")
                            nc.vector.tensor_mul(gc, gcp, mb)
                            ostg = p3.tile([128, LCH], BF16, tag="ostg")
                            eng = (nc.vector, nc.gpsimd, nc.vector)[oc]
                            eng.tensor_add(ostg, x1[:, oc, :], gc)
                            nc.gpsimd.dma_start(
                                out=out_e[b, oc * 128:(oc + 1) * 128,
                                          l0:l0 + LCH],
                                in_=ostg)
    nc.finalize()
    return nc


def kernel(x, t_cond, kernels, D, ada_w, ada_b, w1, b1, w2, b2):
    global _last_results
    consts = _make_consts()
    khre, khim = _make_khat(kernels)
    nc = build_graph()
    shared = {
        "khre": khre,
        "khim": khim,
        "DT": np.ascontiguousarray(
            np.asarray(D, np.float32).reshape(CT, 128).T),
        "ada_wT": np.ascontiguousarray(ada_w.T).astype(ml_dtypes.bfloat16),
        "ada_bT": np.ascontiguousarray(_fold_ada_bias(ada_b)),
        "ada_bR": np.ascontiguousarray(
            _fold_ada_bias(ada_b).T.reshape(1, 18 * 128)
        ).astype(ml_dtypes.bfloat16),
        "w1T": np.ascontiguousarray(w1.T).astype(ml_dtypes.float8_e4m3),
        "b1T": np.ascontiguousarray(
            np.asarray(b1, np.float32).reshape(CT, 128).T),
        "w2T": np.ascontiguousarray(w2.T).astype(ml_dtypes.float8_e4m3),
        "b2T": np.ascontiguousarray(
            np.asarray(b2, np.float32).reshape(CT, 128).T),
    }
    shared.update(consts)
    in_maps = []
    for i in range(NCORES):
        m = dict(shared)
        m["x"] = np.ascontiguousarray(x[i * BPC:(i + 1) * BPC], dtype=np.float32)
        m["t_cond"] = np.ascontiguousarray(t_cond[i * BPC:(i + 1) * BPC],
                                           dtype=np.float32)
        in_maps.append(m)
    trace = os.environ.get("KERNEL_TRACE", "0") == "1"
    res = run_bass_kernel_spmd(nc, in_maps, list(range(NCORES)), trace=trace)
    _last_results = res
    outs = [r["out"] if isinstance(r, dict) else r for r in res.results]
    return np.concatenate([np.asarray(o, dtype=np.float32).reshape(BPC, C, L)
                           for o in outs], axis=0)


if __name__ == "__main__":
    build_graph()
    print("graph built ok")



# revision 53
# speedup vs baseline: 1.3526x; 1.0464x over previous
"""AdaConvBlock Trainium2 kernel: 8-core data-parallel (2 batch elems/core).

Per core (b=2, C=384, L=4096):
  LN1 -> adaLN modulate -> SLConv (1024-tap depthwise conv via four-step
  matmul FFT, N=4608=128x36, batch pair packed as complex) + D-skip ->
  gated residual -> LN2 -> modulate -> pointwise MLP (gelu) -> gated residual.

FFT: Cooley-Tukey twiddles folded into 36 per-n2 stationary matrices ->
pure matmul FFT, no twiddle pointwise work. PSUM is managed in explicit
2KB-bank slots (one matmul output per slot, has_written gives per-slot
overwrite/accumulate semantics).

Key optimizations over the first working version (754us -> 577us CoreSim):
- Kernel FFT precomputed HOST-side (input-dependent only on `kernels`):
  khre/khim DRAM params in the spectral tile layout; removes ~26% of
  phase-2 matmul rows and 12 PSUM->SBUF copies per group.
- Act-table discipline: silu(t_cond) computed once into an SBUF-resident
  tile; phase 3 split into 3a (residual+LN2 stats, Sqrt table) and 3b
  (modulate+MLP, Gelu table); act-table loads dropped 67 -> 4 (-82us).
- scalar_tensor_tensor has NO DVE fast modes (always 1x); ada biases are
  folded into PSUM via K=1 bias matmuls (adabR row layout) so all
  modulate consumers are plain TensorTensor at 2x bf16 rate; PSUM pairs
  drained by Act copies so DVE consumes SBUF bf16.
- LN scale 1/C folded into the ones matrix (ones_c); stats drained as one
  [128,2,L] Act copy; x1 staged via DRAM between ph3a/ph3b; bf16 output.
- fp8e4 weights for the MLP; MLP2 uses DoubleRow perf mode (h produced in
  fp8 by the Gelu for free).
- Large const DMAs (fA/iA) issued mid-phase-1 to overlap transfers.
"""
import os
import sys

sys.path.insert(0, "/opt/trn_rl_repo")

import numpy as np
import ml_dtypes

import concourse.bass as bass
import concourse.bacc as bacc
import concourse.tile as tile
from concourse import mybir
from concourse.bass_utils import run_bass_kernel_spmd

F32 = mybir.dt.float32
BF16 = mybir.dt.bfloat16
AX = mybir.AluOpType
AF = mybir.ActivationFunctionType

B, C, L = 16, 384, 4096
NCORES = 8
BPC = B // NCORES
CT = 3
NCH = 8
LCH = 512
N, N1, N2 = 4608, 128, 36
KLEN = 1024
SS = 512
NG = 8
GS = 48
NSUB = 16
EPS = 1e-5
DECAY = 2.0
NS, KS = 6, 32

_last_results = None


def _make_consts():
    k1 = np.arange(N1)
    n1 = np.arange(N1)
    W128 = np.exp(-2j * np.pi * np.outer(k1, n1) / N1)
    W36 = np.exp(-2j * np.pi * np.outer(np.arange(N2), np.arange(N2)) / N2)
    fA = np.zeros((N2, N1, N1), complex)
    iA = np.zeros((N2, N1, N1), complex)
    for n2 in range(N2):
        M = np.exp(-2j * np.pi * n2 * k1 / N)[:, None] * W128      # [k1,n1]
        fA[n2] = M.T                                               # lhsT [n1,k1]
        IA = (np.exp(2j * np.pi * n2 * k1 / N)[None, :] * np.conj(W128).T) / N
        iA[n2] = IA.T                                              # lhsT [k1,n1]

    def blockdiag3(Mx):
        out = np.zeros((108, 108), complex)
        for s in range(3):
            out[s * 36:(s + 1) * 36, s * 36:(s + 1) * 36] = Mx
        return out

    fB = blockdiag3(W36.T)
    iB = blockdiag3(np.conj(W36).T)

    def bf(x):
        return np.ascontiguousarray(x).astype(ml_dtypes.bfloat16)

    return {
        "fA_re": bf(fA.real), "fA_im": bf(fA.imag), "fA_imn": bf(-fA.imag),
        "iA_re": bf(iA.real), "iA_im": bf(iA.imag), "iA_imn": bf(-iA.imag),
        "fB_re": bf(fB.real), "fB_im": bf(fB.imag), "fB_imn": bf(-fB.imag),
        "iB_re": bf(iB.real), "iB_im": bf(iB.imag), "iB_imn": bf(-iB.imag),
        "ident": bf(np.eye(128)),
        "ones_bf": bf(np.ones((128, 128))),
        "ones_c": bf(np.full((128, 128), 1.0 / C)),
        "ones_row": bf(np.ones((1, 1024))),
    }


def _make_khat(kernels):
    """Host-side kernel FFT: build the normalized multi-scale kernel, FFT to
    length N=4608, and lay out per group in the spectral-domain tile layout
    produced by the on-device fwd FFT: KH[g, c_loc*36+k2, s*128+k1] =
    FFT(k_c)[k1 + 128*k2] with c = 48g + 3s + c_loc."""
    ker = np.asarray(kernels, np.float64)  # (NS, 1, C, KS)
    klist = [np.repeat(ker[i, 0], 2 ** max(0, i - 1), axis=-1)
             * (DECAY ** (NS - i - 1)) for i in range(NS)]
    k = np.concatenate(klist, axis=-1)  # (C, KLEN)
    k = k / np.linalg.norm(k, axis=-1, keepdims=True)
    Ksp = np.fft.fft(k, N, axis=-1)  # (C, N)
    Kp = Ksp.reshape(C, N2, N1)                      # [c, k2, k1]
    Kp = Kp.reshape(NG, GS // 3, 3, N2, N1)          # [g, s, c_loc, k2, k1]
    Kp = Kp.transpose(0, 2, 3, 1, 4).reshape(NG, 108, NSUB * 128)

    def bf(x):
        return np.ascontiguousarray(x).astype(ml_dtypes.bfloat16)

    return bf(Kp.real), bf(Kp.imag)


# bank-slot offset for per-n2 FFT matmul outputs: 10 slots of 48 per 2KB bank
def _n2off(n2):
    return (n2 // 10) * 512 + (n2 % 10) * 48


def _fold_ada_bias(ada_b):
    """[128, 18] bias columns; scale chunks (3..5 tm, 12..14 cm) get +1 so
    modulate is y = z*scale' + shift with scale' = 1 + scale."""
    ab = np.asarray(ada_b, np.float32).reshape(18, 128).T.copy()
    ab[:, 3:6] += 1.0
    ab[:, 12:15] += 1.0
    return ab


def build_graph():
    nc = bacc.Bacc(None)

    x_e = nc.declare_dram_parameter("x", [BPC, C, L], F32, isOutput=False)
    tc_e = nc.declare_dram_parameter("t_cond", [BPC, C // 3, L], F32, isOutput=False)
    khre_e = nc.declare_dram_parameter("khre", [NG, 108, NSUB * 128], BF16,
                                       isOutput=False)
    khim_e = nc.declare_dram_parameter("khim", [NG, 108, NSUB * 128], BF16,
                                       isOutput=False)
    d_e = nc.declare_dram_parameter("DT", [128, CT], F32, isOutput=False)
    adawT_e = nc.declare_dram_parameter("ada_wT", [C // 3, 6 * C], BF16, isOutput=False)
    adab_e = nc.declare_dram_parameter("ada_bT", [128, 18], F32, isOutput=False)
    adabR_e = nc.declare_dram_parameter("ada_bR", [1, 18 * 128], BF16,
                                        isOutput=False)
    FP8 = mybir.dt.float8e4
    w1T_e = nc.declare_dram_parameter("w1T", [C, C], FP8, isOutput=False)
    b1_e = nc.declare_dram_parameter("b1T", [128, CT], F32, isOutput=False)
    w2T_e = nc.declare_dram_parameter("w2T", [C, C], FP8, isOutput=False)
    b2_e = nc.declare_dram_parameter("b2T", [128, CT], F32, isOutput=False)
    cshapes = {
        "fA_re": [N2, N1, N1], "fA_im": [N2, N1, N1], "fA_imn": [N2, N1, N1],
        "iA_re": [N2, N1, N1], "iA_im": [N2, N1, N1], "iA_imn": [N2, N1, N1],
        "fB_re": [108, 108], "fB_im": [108, 108], "fB_imn": [108, 108],
        "iB_re": [108, 108], "iB_im": [108, 108], "iB_imn": [108, 108],
        "ident": [128, 128], "ones_bf": [128, 128],
        "ones_c": [128, 128], "ones_row": [1, 1024],
    }
    cst = {nm: nc.declare_dram_parameter(nm, shp, BF16, isOutput=False)
           for nm, shp in cshapes.items()}
    out_e = nc.declare_dram_parameter("out", [BPC, C, L], BF16, isOutput=True)

    ymod_d = nc.dram_tensor("ymod", [BPC, C, N], BF16)
    yconv_d = nc.dram_tensor("yconv", [BPC, C, N], BF16)
    x1_d = nc.dram_tensor("x1s", [BPC, C, L], BF16)

    MM = nc.tensor.matmul

    with tile.TileContext(nc) as tc, \
         nc.allow_low_precision(reason="bf16 datapath, fp32 psum accumulation"), \
         tc.tile_pool(name="sing", bufs=1) as sing:
        if True:
            zc = sing.tile([128, 1], F32)
            nc.vector.memset(zc, 0.0)
            nc.const_aps.aps[(F32, 0.0)] = zc[:, :]
            ec = sing.tile([128, 1], F32)
            nc.vector.memset(ec, EPS)
            nc.const_aps.aps[(F32, EPS)] = ec[:, :]
            sb = {}
            qengs = (nc.sync, nc.gpsimd, nc.scalar)
            for nm in ("fA_re", "fA_im", "fA_imn", "iA_re", "iA_im", "iA_imn"):
                sb[nm] = sing.tile([N1, N2, N1], BF16, tag=nm, name=nm)
            for nm in ("fB_re", "fB_im", "fB_imn", "iB_re", "iB_im", "iB_imn"):
                sb[nm] = sing.tile([108, 108], BF16, tag=nm, name=nm)
            ident = sing.tile([128, 128], BF16)
            ident32 = sing.tile([128, 128], F32)

            def load_fft_consts():
                # issued mid-ph1: 43us of const DMA overlaps ph1 compute
                for qi, nm in enumerate(("fA_re", "fA_im", "fA_imn", "iA_re",
                                         "iA_im", "iA_imn")):
                    qengs[qi % 3].dma_start(
                        out=sb[nm], in_=cst[nm].rearrange("a b c -> b a c"))
                for qi, nm in enumerate(("fB_re", "fB_im", "fB_imn", "iB_re",
                                         "iB_im", "iB_imn")):
                    qengs[qi % 3].dma_start(out=sb[nm], in_=cst[nm][:, :])
                nc.scalar.dma_start(out=ident, in_=cst["ident"][:, :])
                nc.scalar.activation(ident32, ident, AF.Copy)
            ones_bf = sing.tile([128, 128], BF16)
            nc.sync.dma_start(out=ones_bf, in_=cst["ones_bf"][:, :])
            ones_c = sing.tile([128, 128], BF16)
            nc.gpsimd.dma_start(out=ones_c, in_=cst["ones_c"][:, :])
            ones_row = sing.tile([1, 1024], BF16)
            nc.gpsimd.dma_start(out=ones_row, in_=cst["ones_row"][:, :])
            adabR = sing.tile([1, 18, 128], BF16)
            nc.gpsimd.dma_start(out=adabR,
                                in_=adabR_e.rearrange("a (c o) -> a c o", c=18))

            def bias_mm(psl, ch, lch):
                # K=1 matmul: adds ada bias column for chunk ch across lch
                MM(psl, adabR[0:1, ch, :], ones_row[0:1, 0:lch],
                   start=True, stop=False)
            adawT = sing.tile([128, 18, 128], BF16)
            nc.sync.dma_start(out=adawT,
                              in_=adawT_e.rearrange("k (c o) -> k c o", c=18))
            adab = sing.tile([128, 18], F32)
            nc.sync.dma_start(out=adab, in_=adab_e[:, :])
            w1T = sing.tile([128, CT, C], FP8)
            nc.sync.dma_start(out=w1T,
                              in_=w1T_e.rearrange("(a k) o -> k a o", k=128))
            w2T = sing.tile([128, CT, C], FP8)
            nc.sync.dma_start(out=w2T,
                              in_=w2T_e.rearrange("(a k) o -> k a o", k=128))
            b1c = sing.tile([128, CT], F32)
            nc.sync.dma_start(out=b1c, in_=b1_e[:, :])
            b2c = sing.tile([128, CT], F32)
            nc.sync.dma_start(out=b2c, in_=b2_e[:, :])
            dcol = sing.tile([128, CT], F32)
            nc.sync.dma_start(out=dcol, in_=d_e[:, :])

            # silu(t_cond) computed once, resident for ph1 (ada) and ph3
            tsl = sing.tile([128, BPC, L], BF16, tag="tsl")
            with tc.tile_pool(name="p0", bufs=2) as p0:
                for b in range(BPC):
                    tcf = p0.tile([128, L], F32, tag="tcf")
                    qengs[2 if b == 0 else 1].dma_start(out=tcf,
                                                        in_=tc_e[b, :, :])
                    nc.scalar.activation(tsl[:, b, :], tcf, AF.Silu)

            import os as _os
            _STAGES = int(_os.environ.get("KSTAGES", "4"))
            # ---------------- phase 1: LN1 + modulate -> ymod -------
            # NOTE: ada_bT scale chunks (3..5, 12..14) carry a host-folded +1
            # so modulate is y = z*(scale') + shift with scale' = 1+scale.
            if _STAGES >= 2:
             with tc.tile_pool(name="p1", bufs=3) as p1, \
                 tc.tile_pool(name="p1p", bufs=1, space="PSUM") as p1p, \
                 tc.tile_pool(name="p1q", bufs=3, space="PSUM") as p1q:
                zpadN = p1.tile([128, N - L], BF16, tag="zpadN")
                nc.vector.memset(zpadN, 0.0)
                for b in range(BPC):
                    for ct in range(CT):
                        nc.gpsimd.dma_start(
                            out=ymod_d[b, ct * 128:(ct + 1) * 128, L:N],
                            in_=zpadN)
                for b in range(BPC):
                    for ch in range(NCH):
                        if b == 0 and ch == 2:
                            load_fft_consts()
                        l0 = ch * LCH
                        xf = p1.tile([128, CT, LCH], F32, tag="xf")
                        for ct in range(CT):
                            dq = nc.sync if ct != 1 else nc.gpsimd
                            dq.dma_start(
                                out=xf[:, ct, :],
                                in_=x_e[b, ct * 128:(ct + 1) * 128, l0:l0 + LCH])
                        xb = p1.tile([128, CT, LCH], BF16, tag="xb")
                        x2 = p1.tile([128, CT, LCH], BF16, tag="x2")
                        for ct in range(CT):
                            if ct == 2:
                                nc.vector.tensor_copy(xb[:, ct, :], xf[:, ct, :])
                            else:
                                nc.scalar.activation(xb[:, ct, :], xf[:, ct, :],
                                                     AF.Copy)
                            eng = nc.vector if ct != 0 else nc.gpsimd
                            eng.tensor_mul(x2[:, ct, :], xb[:, ct, :],
                                           xb[:, ct, :])
                        sst = p1p.tile([128, 2, LCH], F32, tag="sst")
                        for ct in range(CT):
                            MM(sst[:, 0, :], ones_c, xb[:, ct, :],
                               start=(ct == 0), stop=(ct == CT - 1))
                        for ct in range(CT):
                            MM(sst[:, 1, :], ones_c, x2[:, ct, :],
                               start=(ct == 0), stop=(ct == CT - 1))
                        muex = p1.tile([128, 2, LCH], BF16, tag="muex")
                        nc.scalar.activation(muex, sst, AF.Copy)
                        mu, ex = muex[:, 0, :], muex[:, 1, :]
                        musq = p1.tile([128, LCH], BF16, tag="musq")
                        nc.gpsimd.tensor_mul(musq, mu, mu)
                        var = p1.tile([128, LCH], BF16, tag="var")
                        nc.vector.tensor_sub(var, ex, musq)
                        sd_ = p1.tile([128, LCH], F32, tag="sd_")
                        nc.scalar.activation(sd_, var, AF.Sqrt, bias=EPS)
                        inv = p1.tile([128, LCH], BF16, tag="inv")
                        nc.vector.reciprocal(inv, sd_)
                        muinv = p1.tile([128, LCH], BF16, tag="muinv")
                        nc.vector.tensor_mul(muinv, mu, inv)
                        ym = p1.tile([128, CT, LCH], BF16, tag="ym")
                        for ct in range(CT):
                            adp = p1q.tile([128, 2, LCH], F32, tag="adp")
                            bias_mm(adp[:, 0, :], ct, LCH)
                            MM(adp[:, 0, :], adawT[:, ct, :],
                               tsl[:, b, l0:l0 + LCH], start=False, stop=True)
                            bias_mm(adp[:, 1, :], 3 + ct, LCH)
                            MM(adp[:, 1, :], adawT[:, 3 + ct, :],
                               tsl[:, b, l0:l0 + LCH], start=False, stop=True)
                            m1 = p1.tile([128, LCH], BF16, tag=f"m1_{ct}")
                            eng = nc.vector if ct != 1 else nc.gpsimd
                            eng.tensor_mul(m1, xb[:, ct, :], inv)
                            z = p1.tile([128, LCH], BF16, tag=f"z_{ct}")
                            eng2 = nc.gpsimd if ct != 1 else nc.vector
                            eng2.tensor_sub(z, m1, muinv)
                            u = p1.tile([128, LCH], BF16, tag=f"u_{ct}")
                            if ct != 1:
                                # drain biased PSUM via Act copy; TT at 2x
                                adb = p1.tile([128, 2, LCH], BF16,
                                              tag=f"adb_{ct}")
                                nc.scalar.activation(adb, adp, AF.Copy)
                                srcS, srcSH = adb[:, 1, :], adb[:, 0, :]
                            else:
                                srcS, srcSH = adp[:, 1, :], adp[:, 0, :]
                            nc.vector.tensor_mul(u, srcS, z)
                            nc.vector.tensor_add(ym[:, ct, :], srcSH, u)
                        for ct in range(CT):
                            nc.gpsimd.dma_start(
                                out=ymod_d[b, ct * 128:(ct + 1) * 128,
                                           l0:l0 + LCH],
                                in_=ym[:, ct, :])

            tc.strict_bb_all_engine_barrier()
            # ---------------- phase 2: kernel FFT + conv FFT --------
            if _STAGES >= 3:
             with tc.tile_pool(name="p2", bufs=1) as p2, \
                 tc.tile_pool(name="p2in", bufs=2) as p2in, \
                 tc.tile_pool(name="p2p", bufs=1, space="PSUM") as p2p:

                H2 = [(0, 20), (20, 16)]    # n2 halves (start, count)
                HS = [(0, 8), (8, 8)]        # subgroup halves

                def pair(h):
                    t = "a" if h == 0 else "b"
                    pre = p2p.tile([128, 1024], F32, tag=f"p{t}_re")
                    pim = p2p.tile([128, 1024], F32, tag=f"p{t}_im")
                    return pre, pim

                def unscr_half(dst, psrc, h, eng):
                    # psrc [128,1024]: 2 banks of 10 slots x 48 -> dst ch-major
                    d3 = dst.rearrange("p (c n) -> p c n", n=N2)
                    st, cnt = H2[h]
                    s5 = psrc.rearrange("p (bk r) -> p bk r", bk=2)[
                        :, :, 0:480].rearrange("p bk (sl c) -> p bk sl c",
                                               sl=10)
                    act = eng is nc.scalar
                    if cnt == 20:
                        o = d3[:, :, st:st + 20].rearrange(
                            "p c (bk sl) -> p bk sl c", bk=2)
                        if act:
                            eng.activation(o, s5[:, :, :, 0:48], AF.Copy)
                        else:
                            eng.tensor_copy(o, s5[:, :, :, 0:48])
                    else:
                        o1 = d3[:, :, st:st + 10].rearrange("p c n -> p n c")
                        o2 = d3[:, :, st + 10:st + 16].rearrange(
                            "p c n -> p n c")
                        if act:
                            eng.activation(o1, s5[:, 0, :, 0:48], AF.Copy)
                            eng.activation(o2, s5[:, 1, 0:6, 0:48], AF.Copy)
                        else:
                            eng.tensor_copy(o1, s5[:, 0, :, 0:48])
                            eng.tensor_copy(o2, s5[:, 1, 0:6, 0:48])

                def f1_half(pre, pim, h, zr, zi, real):
                    st, cnt = H2[h]
                    for jx in range(cnt):
                        n2 = st + jx
                        off = (jx // 10) * 512 + (jx % 10) * 48
                        if real:
                            MM(pre[:, off:off + GS], sb["fA_re"][0:29, n2, :],
                               zr[:, :, n2], start=True, stop=True)
                            MM(pim[:, off:off + GS], sb["fA_im"][0:29, n2, :],
                               zr[:, :, n2], start=True, stop=True)
                        else:
                            MM(pre[:, off:off + GS], sb["fA_re"][:, n2, :],
                               zr[:, :, n2], start=True, stop=False)
                            MM(pim[:, off:off + GS], sb["fA_im"][:, n2, :],
                               zr[:, :, n2], start=True, stop=False)
                            MM(pre[:, off:off + GS], sb["fA_imn"][:, n2, :],
                               zi[:, :, n2], start=False, stop=True)
                            MM(pim[:, off:off + GS], sb["fA_re"][:, n2, :],
                               zi[:, :, n2], start=False, stop=True)

                def i4_half(pre, pim, h, vr3, vi3):
                    st, cnt = H2[h]
                    for jx in range(cnt):
                        n2 = st + jx
                        off = (jx // 10) * 512 + (jx % 10) * 48
                        MM(pre[:, off:off + GS], sb["iA_re"][:, n2, :],
                           vr3[:, :, n2], start=True, stop=False)
                        MM(pim[:, off:off + GS], sb["iA_im"][:, n2, :],
                           vr3[:, :, n2], start=True, stop=False)
                        MM(pre[:, off:off + GS], sb["iA_imn"][:, n2, :],
                           vi3[:, :, n2], start=False, stop=True)
                        MM(pim[:, off:off + GS], sb["iA_re"][:, n2, :],
                           vi3[:, :, n2], start=False, stop=True)

                def t_half(pre, pim, h, inre, inim):
                    st, _ = HS[h]
                    for s in range(st, st + 8):
                        off = ((s - st) // 4) * 512 + ((s - st) % 4) * 128
                        isl = slice(s * 108, (s + 1) * 108)
                        MM(pre[:108, off:off + 128], inre[:, isl], ident32,
                           is_transpose=True, start=True, stop=True)
                        MM(pim[:108, off:off + 128], inim[:, isl], ident32,
                           is_transpose=True, start=True, stop=True)

                def tb_half(pre, pim, h, inre, inim):
                    st, _ = HS[h]
                    for s in range(st, st + 8):
                        off = ((s - st) // 4) * 512 + ((s - st) % 4) * 108
                        isl = slice(s * 128, (s + 1) * 128)
                        MM(pre[:, off:off + 108], inre[:108, isl],
                           ident32[:108, :108], is_transpose=True, start=True,
                           stop=True)
                        MM(pim[:, off:off + 108], inim[:108, isl],
                           ident32[:108, :108], is_transpose=True, start=True,
                           stop=True)

                def d36_half(pre, pim, h, Bre, Bim, Bimn, inre, inim):
                    st, _ = HS[h]
                    for s in range(st, st + 8):
                        off = (s - st) * 128
                        sl = slice(s * 128, (s + 1) * 128)
                        MM(pre[:108, off:off + 128], Bre, inre[:, sl],
                           start=True, stop=False)
                        MM(pim[:108, off:off + 128], Bim, inre[:, sl],
                           start=True, stop=False)
                        MM(pre[:108, off:off + 128], Bimn, inim[:, sl],
                           start=False, stop=True)
                        MM(pim[:108, off:off + 128], Bre, inim[:, sl],
                           start=False, stop=True)

                def hcopy(dst, psrc, h, eng):
                    o = dst[:, h * 1024:(h + 1) * 1024]
                    if eng is nc.scalar:
                        eng.activation(o, psrc[:108, :], AF.Copy)
                    else:
                        eng.tensor_copy(o, psrc[:108, :])

                def vcopy_half(vflat, psrc, h, eng):
                    for bk in range(2):
                        o = vflat[:, (h * 8 + bk * 4) * 108:
                                  (h * 8 + bk * 4) * 108 + 432]
                        s_ = psrc[:, bk * 512:bk * 512 + 432]
                        if eng is nc.scalar:
                            eng.activation(o, s_, AF.Copy)
                        else:
                            eng.tensor_copy(o, s_)

                def fwd_group(g):
                    # forward FFT of group g: loads -> f1 -> t -> d36 -> X
                    c0 = g * GS
                    kh_re = p2in.tile([108, NSUB * 128], BF16, tag="kh_re",
                                      name="kh_re")
                    kh_im = p2in.tile([108, NSUB * 128], BF16, tag="kh_im",
                                      name="kh_im")
                    nc.gpsimd.dma_start(out=kh_re, in_=khre_e[g])
                    nc.gpsimd.dma_start(out=kh_im, in_=khim_e[g])
                    z_re = p2in.tile([128, GS, N2], BF16, tag="z_re",
                                     name="z_re")
                    z_im = p2in.tile([128, GS, N2], BF16, tag="z_im",
                                     name="z_im")
                    nc.sync.dma_start(
                        out=z_re, in_=ymod_d[0, c0:c0 + GS, :].rearrange(
                            "c (a b) -> a c b", b=N2))
                    nc.sync.dma_start(
                        out=z_im, in_=ymod_d[1, c0:c0 + GS, :].rearrange(
                            "c (a b) -> a c b", b=N2))
                    S_re = p2.tile([128, GS * N2], F32, tag="S_re",
                                   name="S_re")
                    S_im = p2.tile([128, GS * N2], F32, tag="S_im",
                                   name="S_im")
                    for h in (0, 1):
                        pre, pim = pair(h)
                        f1_half(pre, pim, h, z_re, z_im, False)
                        unscr_half(S_re, pre, h, nc.scalar)
                        unscr_half(S_im, pim, h, nc.vector)
                    ST_re = p2.tile([108, NSUB * 128], BF16, tag="ST_re",
                                    name="ST_re")
                    ST_im = p2.tile([108, NSUB * 128], BF16, tag="ST_im",
                                    name="ST_im")
                    for h in (0, 1):
                        pre, pim = pair(h)
                        t_half(pre, pim, h, S_re, S_im)
                        hcopy(ST_re, pre, h, nc.scalar)
                        hcopy(ST_im, pim, h, nc.vector)
                    # X double-buffered by parity: mult(g) overlaps fwd(g+1)
                    X_re = p2.tile([108, NSUB * 128], BF16,
                                   tag=f"X_re{g % 2}", name="X_re")
                    X_im = p2.tile([108, NSUB * 128], BF16,
                                   tag=f"X_im{g % 2}", name="X_im")
                    for h in (0, 1):
                        pre, pim = pair(h)
                        d36_half(pre, pim, h, sb["fB_re"], sb["fB_im"],
                                 sb["fB_imn"], ST_re, ST_im)
                        hcopy(X_re, pre, h, nc.scalar)
                        hcopy(X_im, pim, h, nc.vector)
                    return X_re, X_im, kh_re, kh_im

                def mult_group(g, X_re, X_im, kh_re, kh_im):
                    # spectral multiply: runs on DVE/Pool during fwd(g+1)
                    Y_re = p2.tile([108, NSUB * 128], BF16, tag="Y_re",
                                   name="Y_re")
                    Y_im = p2.tile([108, NSUB * 128], BF16, tag="Y_im",
                                   name="Y_im")
                    q1 = p2.tile([108, NSUB * 128], BF16, tag="q1", name="q1")
                    q2 = p2.tile([108, NSUB * 128], BF16, tag="q2", name="q2")
                    for h in (0, 1):
                        sl = slice(h * 1024, (h + 1) * 1024)
                        nc.vector.tensor_mul(q1[:, sl], X_re[:, sl],
                                             kh_re[:, sl])
                        nc.gpsimd.tensor_mul(q2[:, sl], X_im[:, sl],
                                             kh_im[:, sl])
                        nc.gpsimd.tensor_sub(Y_re[:, sl], q1[:, sl],
                                             q2[:, sl])
                        nc.vector.tensor_mul(q1[:, sl], X_re[:, sl],
                                             kh_im[:, sl])
                        nc.gpsimd.tensor_mul(q2[:, sl], X_im[:, sl],
                                             kh_re[:, sl])
                        nc.vector.tensor_add(Y_im[:, sl], q1[:, sl],
                                             q2[:, sl])
                    return Y_re, Y_im

                def inv_group(g, Y_re, Y_im):
                    c0 = g * GS
                    U_re = p2.tile([108, NSUB * 128], F32, tag="U_re",
                                   name="U_re")
                    U_im = p2.tile([108, NSUB * 128], F32, tag="U_im",
                                   name="U_im")
                    for h in (0, 1):
                        pre, pim = pair(h)
                        d36_half(pre, pim, h, sb["iB_re"], sb["iB_im"],
                                 sb["iB_imn"], Y_re, Y_im)
                        hcopy(U_re, pre, h, nc.scalar)
                        hcopy(U_im, pim, h, nc.vector)
                    V_re = p2.tile([128, GS, N2], BF16, tag="V_re",
                                   name="V_re")
                    V_im = p2.tile([128, GS, N2], BF16, tag="V_im",
                                   name="V_im")
                    vr = V_re.rearrange("p a b -> p (a b)")
                    vi = V_im.rearrange("p a b -> p (a b)")
                    for h in (0, 1):
                        pre, pim = pair(h)
                        tb_half(pre, pim, h, U_re, U_im)
                        vcopy_half(vr, pre, h, nc.scalar)
                        vcopy_half(vi, pim, h, nc.vector)
                    yo_re = p2.tile([128, GS, N2], BF16, tag="yo_re",
                                    name="yo_re")
                    yo_im = p2.tile([128, GS, N2], BF16, tag="yo_im",
                                    name="yo_im")
                    yof_re = yo_re.rearrange("p a b -> p (a b)")
                    yof_im = yo_im.rearrange("p a b -> p (a b)")
                    for h in (0, 1):
                        pre, pim = pair(h)
                        i4_half(pre, pim, h, V_re, V_im)
                        unscr_half(yof_re, pre, h, nc.scalar)
                        unscr_half(yof_im, pim, h, nc.vector)
                    nc.gpsimd.dma_start(
                        out=yconv_d[0, c0:c0 + GS, :].rearrange(
                            "c (a b) -> a c b", b=N2), in_=yo_re)
                    nc.gpsimd.dma_start(
                        out=yconv_d[1, c0:c0 + GS, :].rearrange(
                            "c (a b) -> a c b", b=N2), in_=yo_im)

                # software pipeline: PE runs fwd(g+1) while DVE/Pool run
                # mult(g); then inv(g) finds Y ready -> no PE stall.
                prevX = None
                for g in range(NG):
                    if prevX is not None:
                        Yp = mult_group(g - 1, *prevX)
                    X = fwd_group(g)
                    if prevX is not None:
                        inv_group(g - 1, *[Yp[0], Yp[1]])
                    prevX = X
                Yp = mult_group(NG - 1, *prevX)
                inv_group(NG - 1, Yp[0], Yp[1])

            tc.strict_bb_all_engine_barrier()
            # ------- phase 3a: residual + gate_tm + LN2 stats (Rsqrt) -------
            # inv2/muinv2 kept SBUF-resident for ph3b; x1 staged via DRAM.
            if _STAGES >= 4:
             with tc.tile_pool(name="p3r", bufs=1) as p3r:
              inv2r = p3r.tile([128, BPC, L], BF16, tag="inv2r")
              muinv2r = p3r.tile([128, BPC, L], BF16, tag="muinv2r")
              with tc.tile_pool(name="p3a", bufs=2) as p3, \
                  tc.tile_pool(name="p3ap", bufs=1, space="PSUM") as p3p, \
                  tc.tile_pool(name="p3aq", bufs=3, space="PSUM") as p3pm:
                for ch in range(NCH):
                    for b in range(BPC):
                        l0 = ch * LCH
                        yc = p3.tile([128, CT, LCH], BF16, tag="yc3")
                        ym3 = p3.tile([128, CT, LCH], BF16, tag="ym3")
                        xf3 = p3.tile([128, CT, LCH], F32, tag="xf3")
                        for ct in range(CT):
                            nc.sync.dma_start(
                                out=xf3[:, ct, :],
                                in_=x_e[b, ct * 128:(ct + 1) * 128, l0:l0 + LCH])
                            nc.sync.dma_start(
                                out=yc[:, ct, :],
                                in_=yconv_d[b, ct * 128:(ct + 1) * 128,
                                            SS + l0:SS + l0 + LCH])
                            nc.gpsimd.dma_start(
                                out=ym3[:, ct, :],
                                in_=ymod_d[b, ct * 128:(ct + 1) * 128,
                                           l0:l0 + LCH])
                        x1 = p3.tile([128, CT, LCH], BF16, tag="x1")
                        x2t = p3.tile([128, CT, LCH], BF16, tag="x2t")
                        for ct in range(CT):
                            adp3 = p3pm.tile([128, LCH], F32, tag="adp3")
                            bias_mm(adp3, 6 + ct, LCH)
                            MM(adp3, adawT[:, 6 + ct, :],
                               tsl[:, b, l0:l0 + LCH], start=False, stop=True)
                            # s1 = D*ym + yconv: tensor_scalar (4x) + TT (2x)
                            dm = p3.tile([128, LCH], BF16, tag=f"dm_{ct}")
                            nc.vector.tensor_scalar(
                                dm, ym3[:, ct, :], dcol[:, ct:ct + 1],
                                None, AX.mult)
                            s1 = p3.tile([128, LCH], BF16, tag=f"s1_{ct}")
                            eng0 = nc.gpsimd if ct == 1 else nc.vector
                            eng0.tensor_add(s1, dm, yc[:, ct, :])
                            # gx = gate_tm' * s1 (bias folded into PSUM)
                            gx = p3.tile([128, LCH], BF16, tag=f"gx_{ct}")
                            if ct != 1:
                                gtb = p3.tile([128, LCH], BF16, tag=f"gtb_{ct}")
                                nc.scalar.activation(gtb, adp3, AF.Copy)
                                nc.vector.tensor_mul(gx, gtb, s1)
                            else:
                                nc.vector.tensor_mul(gx, adp3, s1)
                            eng = nc.vector if ct == 1 else nc.gpsimd
                            eng.tensor_add(x1[:, ct, :], xf3[:, ct, :], gx)
                            nc.scalar.activation(x2t[:, ct, :], x1[:, ct, :],
                                                 AF.Square)
                            nc.gpsimd.dma_start(
                                out=x1_d[b, ct * 128:(ct + 1) * 128,
                                         l0:l0 + LCH],
                                in_=x1[:, ct, :])
                        sst3 = p3p.tile([128, 2, LCH], F32, tag="sst3")
                        for ct in range(CT):
                            MM(sst3[:, 0, :], ones_c, x1[:, ct, :],
                               start=(ct == 0), stop=(ct == CT - 1))
                        for ct in range(CT):
                            MM(sst3[:, 1, :], ones_c, x2t[:, ct, :],
                               start=(ct == 0), stop=(ct == CT - 1))
                        muex3 = p3.tile([128, 2, LCH], BF16, tag="muex3")
                        nc.scalar.activation(muex3, sst3, AF.Copy)
                        mu, ex3 = muex3[:, 0, :], muex3[:, 1, :]
                        musq = p3.tile([128, LCH], BF16, tag="musq3")
                        nc.gpsimd.tensor_mul(musq, mu, mu)
                        var = p3.tile([128, LCH], BF16, tag="var3")
                        nc.vector.tensor_sub(var, ex3, musq)
                        sd3 = p3.tile([128, LCH], F32, tag="sd3")
                        nc.scalar.activation(sd3, var, AF.Sqrt, bias=EPS)
                        nc.vector.reciprocal(inv2r[:, b, l0:l0 + LCH], sd3)
                        nc.vector.tensor_mul(muinv2r[:, b, l0:l0 + LCH], mu,
                                             inv2r[:, b, l0:l0 + LCH])

              tc.strict_bb_all_engine_barrier()
              # ------- phase 3b: modulate_cm + MLP (Gelu) + gated out -------
              with tc.tile_pool(name="p3b", bufs=2) as p3, \
                  tc.tile_pool(name="p3bq", bufs=2, space="PSUM") as p3q, \
                  tc.tile_pool(name="p3bm", bufs=2, space="PSUM") as p3m, \
                  tc.tile_pool(name="p3bg", bufs=2, space="PSUM") as p3g:
                def z2_stage(b, l0):
                    # loads x1, computes modulated z2 (DVE/Act heavy)
                    x1 = p3.tile([128, CT, LCH], BF16, tag="x1b", name="x1")
                    for ct in range(CT):
                        nc.sync.dma_start(
                            out=x1[:, ct, :],
                            in_=x1_d[b, ct * 128:(ct + 1) * 128,
                                     l0:l0 + LCH])
                    inv = inv2r[:, b, l0:l0 + LCH]
                    muinv = muinv2r[:, b, l0:l0 + LCH]
                    z2 = p3.tile([128, CT, LCH], BF16, tag="z2", name="z2")
                    for ct in range(CT):
                        adp = p3q.tile([128, 2, LCH], F32, tag="adp",
                                       name="adp")
                        bias_mm(adp[:, 0, :], 9 + ct, LCH)
                        MM(adp[:, 0, :], adawT[:, 9 + ct, :],
                           tsl[:, b, l0:l0 + LCH], start=False, stop=True)
                        bias_mm(adp[:, 1, :], 12 + ct, LCH)
                        MM(adp[:, 1, :], adawT[:, 12 + ct, :],
                           tsl[:, b, l0:l0 + LCH], start=False, stop=True)
                        m1 = p3.tile([128, LCH], BF16, tag=f"m13_{ct}",
                                     name="m1")
                        eng = nc.vector if ct != 1 else nc.gpsimd
                        eng.tensor_mul(m1, x1[:, ct, :], inv)
                        z = p3.tile([128, LCH], BF16, tag=f"z3_{ct}",
                                    name="z")
                        eng2 = nc.gpsimd if ct != 1 else nc.vector
                        eng2.tensor_sub(z, m1, muinv)
                        u = p3.tile([128, LCH], BF16, tag=f"u3_{ct}",
                                    name="u")
                        if ct != 1:
                            adb = p3.tile([128, 2, LCH], BF16,
                                          tag=f"adb3_{ct}", name="adb")
                            nc.scalar.activation(adb, adp, AF.Copy)
                            srcS, srcSH = adb[:, 1, :], adb[:, 0, :]
                        else:
                            srcS, srcSH = adp[:, 1, :], adp[:, 0, :]
                        nc.vector.tensor_mul(u, srcS, z)
                        nc.vector.tensor_add(z2[:, ct, :], srcSH, u)
                    return x1, z2, b, l0

                def mlp_stage(x1, z2, b, l0):
                    h = p3.tile([128, CT, LCH], FP8, tag="h", name="h")
                    for oc in range(CT):
                        hp = p3m.tile([128, LCH], F32, tag="mlp", name="hp")
                        for ct in range(CT):
                            MM(hp, w1T[:, ct, oc * 128:(oc + 1) * 128],
                               z2[:, ct, :], start=(ct == 0),
                               stop=(ct == CT - 1))
                        nc.scalar.activation(h[:, oc, :], hp, AF.Gelu,
                                             bias=b1c[:, oc:oc + 1])
                    for oc in range(CT):
                        gcp = p3g.tile([128, LCH], F32, tag="gcs", name="gcp")
                        bias_mm(gcp, 15 + oc, LCH)
                        MM(gcp, adawT[:, 15 + oc, :],
                           tsl[:, b, l0:l0 + LCH], start=False, stop=True)
                        mp = p3m.tile([128, LCH], F32, tag="mlp", name="mp")
                        MM(mp, w2T[:, 0:2, oc * 128:(oc + 1) * 128],
                           h[:, 0:2, :], start=True, stop=False,
                           perf_mode=mybir.MatmulPerfMode.DoubleRow)
                        MM(mp, w2T[:, 2, oc * 128:(oc + 1) * 128],
                           h[:, 2, :], start=False, stop=True)
                        mb = p3.tile([128, LCH], BF16, tag=f"mb_{oc}",
                                     name="mb")
                        nc.scalar.activation(mb, mp, AF.Identity,
                                             bias=b2c[:, oc:oc + 1])
                        gc = p3.tile([128, LCH], BF16, tag=f"gc_{oc}",
                                     name="gc")
                        nc.vector.tensor_mul(gc, gcp, mb)
                        ostg = p3.tile([128, LCH], BF16, tag="ostg",
                                       name="ostg")
                        eng = (nc.vector, nc.gpsimd, nc.vector)[oc]
                        eng.tensor_add(ostg, x1[:, oc, :], gc)
                        nc.gpsimd.dma_start(
                            out=out_e[b, oc * 128:(oc + 1) * 128,
                                      l0:l0 + LCH],
                            in_=ostg)

                for ch in range(NCH):
                    for b in range(BPC):
                        mlp_stage(*z2_stage(b, ch * LCH))
    nc.finalize()
    return nc


def kernel(x, t_cond, kernels, D, ada_w, ada_b, w1, b1, w2, b2):
    global _last_results
    consts = _make_consts()
    khre, khim = _make_khat(kernels)
    nc = build_graph()
    shared = {
        "khre": khre,
        "khim": khim,
        "DT": np.ascontiguousarray(
            np.asarray(D, np.float32).reshape(CT, 128).T),
        "ada_wT": np.ascontiguousarray(ada_w.T).astype(ml_dtypes.bfloat16),
        "ada_bT": np.ascontiguousarray(_fold_ada_bias(ada_b)),
        "ada_bR": np.ascontiguousarray(
            _fold_ada_bias(ada_b).T.reshape(1, 18 * 128)
        ).astype(ml_dtypes.bfloat16),
        "w1T": np.ascontiguousarray(w1.T).astype(ml_dtypes.float8_e4m3),
        "b1T": np.ascontiguousarray(
            np.asarray(b1, np.float32).reshape(CT, 128).T),
        "w2T": np.ascontiguousarray(w2.T).astype(ml_dtypes.float8_e4m3),
        "b2T": np.ascontiguousarray(
            np.asarray(b2, np.float32).reshape(CT, 128).T),
    }
    shared.update(consts)
    in_maps = []
    for i in range(NCORES):
        m = dict(shared)
        m["x"] = np.ascontiguousarray(x[i * BPC:(i + 1) * BPC], dtype=np.float32)
        m["t_cond"] = np.ascontiguousarray(t_cond[i * BPC:(i + 1) * BPC],
                                           dtype=np.float32)
        in_maps.append(m)
    trace = os.environ.get("KERNEL_TRACE", "0") == "1"
    res = run_bass_kernel_spmd(nc, in_maps, list(range(NCORES)), trace=trace)
    _last_results = res
    outs = [r["out"] if isinstance(r, dict) else r for r in res.results]
    return np.concatenate([np.asarray(o, dtype=np.float32).reshape(BPC, C, L)
                           for o in outs], axis=0)


if __name__ == "__main__":
    build_graph()
    print("graph built ok")

